# revision 68
# baseline (speedup 1.0000x reference)
"""Trainium2 Bass kernel for MultiHead GQA attention (B=2, S=2048, D=2048,
H=16 query heads, HKV=4 kv heads, DH=128, RoPE, mask, out-proj).

Sharding: token-parallel across 8 cores. Core c handles batch c//4 and 512
query rows of it. Each core projects K/V for its own 512-token quarter
(all 4 kv heads), the quarters are all-gathered in 4 pipelined 128-token
chunks, and the core runs attention + out-proj for its rows. Host
reassembles. All matmuls bf16 with fp32 PSUM accumulation.

Causal handling (exact, SPMD-uniform): core r of its batch owns the 16
interleaved 32-row q-blocks {4j + r : j=0..15} (ascending). For key tile
kc the q-blocks that attend to it are exactly the suffix of blocks with
position j >= kc, i.e. a contiguous column suffix of width n = 32*(16-kc)
-- identical on every core. Only the first 32 columns of each suffix (the
diagonal block) are partially masked; they get multiplied by a per-core
[128, 32] 0/1 tile. This computes 34 128x128-tile-equivalents per head
(the exact causal minimum for a 4-way row split) vs 40 for the previous
128-row-block scheme.

Attention is computed transposed: scoresT[keys, q] = khT.T @ qhT per
128-key tile, exp on ScalarE (scale folded in), probs bf16, then
outT[dh, q] += v_tile.T @ probsT, and row-sums via a ones-stationary
matmul. outT feeds the out-projection directly as stationary operand.

The K/V all-gather is split into 4 collectives, one per 128-token block
of each rank's quarter: chunk m delivers key tiles {4r + m : r=0..3}.
Attention iterates kc in the order [0,4,8,12, 1,5,9,13, ...] so the
first kc group only needs chunk 0 -- the remaining chunks stream in
behind attention/Q-proj compute instead of serializing in front of it.

Mask modes (host-detected, compile-time): none / causal / mask as before;
"mask" computes the full rectangle (n=512) and multiplies by the 0/1 mask.
"""

import math

import numpy as np
import ml_dtypes

import concourse.bass as bass
import concourse.mybir as mybir
import concourse.tile as tile
from concourse import bacc
from concourse.bass_utils import run_bass_kernel_spmd

F32 = mybir.dt.float32
BF16 = mybir.dt.bfloat16
BF = ml_dtypes.bfloat16

B, S, D = 2, 2048, 2048
H, G = 16, 4
HKV = H // G            # 4
DH = D // H             # 128
DKV = D // G            # 512 (kv projection width)
NCORES = 8
RPC = S // 4            # 512 rows per core
NIC = D // 128          # 16 contraction chunks
NKC = S // 128          # 16 key tiles
SCALE = 1.0 / math.sqrt(DH)
# attention kc order: the K/V all-gather is split in 2 chunks; chunk A
# carries 128-token blocks {0,1} of each rank's quarter (= key tiles
# {4r, 4r+1}), chunk B blocks {2,3}.  Attention runs in two phases: an
# A pass over all head pairs (chunk-A tiles only, partial sums saved to
# SBUF), then a B pass -- so the A pass needs only chunk A, and chunk B
# arrives long before the B pass starts.
CHUNK_BLKS = [(0, 2), (2, 4)]        # [lo, hi) own-token-block range per chunk
KC_A = [4 * b + m for b in range(4) for m in range(2)]
KC_B = [4 * b + m for b in range(4) for m in range(2, 4)]

_NC_CACHE: dict = {}

# set by callers (e.g. test.py) to capture a profile; results of the last run
TRACE = False
TRACE_CORES = None          # e.g. [0] or list(range(8))
LAST_RESULTS = None


def _n_list(mode: str) -> list[int]:
    """Moving-operand width (in q columns, suffix of the 512) per key tile."""
    if mode == "causal":
        return [32 * (16 - kc) for kc in range(NKC)]
    return [512] * NKC


def _build(mode: str):
    mask_mul = mode != "none"
    n_of = _n_list(mode)

    nc = bacc.Bacc("TRN2", target_bir_lowering=False, debug=False,
                   num_devices=NCORES)

    # ---- I/O (host-prepared layouts; all contiguous-DMA friendly) ----
    wq = nc.declare_dram_parameter("wq", [NIC, 128, D], BF16, isOutput=False)
    qt = nc.declare_dram_parameter("qt", [128, NIC * RPC], BF16, isOutput=False)
    # k/v: only this core's 512-token quarter (projected here, all-gathered)
    kt = nc.declare_dram_parameter("kt", [128, NIC * 512], BF16, isOutput=False)
    vt = nc.declare_dram_parameter("vt", [4, 128, NIC * 128], BF16, isOutput=False)
    wk = nc.declare_dram_parameter("wk", [HKV, 128, NIC * 128], BF16, isOutput=False)
    wv = nc.declare_dram_parameter("wv", [128, NIC * DKV], BF16, isOutput=False)
    wo = nc.declare_dram_parameter("wo", [4, 128, H * 512], BF16, isOutput=False)
    cosq = nc.declare_dram_parameter("cosq", [128, RPC], BF16, isOutput=False)
    sinq = nc.declare_dram_parameter("sinq", [128, RPC], BF16, isOutput=False)
    # cos/sin for this core's own k-token quarter
    cosk = nc.declare_dram_parameter("cosk", [128, 512], BF16, isOutput=False)
    sink = nc.declare_dram_parameter("sink", [128, 512], BF16, isOutput=False)
    pswap = nc.declare_dram_parameter("pswap", [128, 128], BF16, isOutput=False)
    if mode == "causal":
        mdiag = nc.declare_dram_parameter("mdiag", [128, 32], BF16,
                                          isOutput=False)
    if mode == "mask":
        m01 = nc.declare_dram_parameter("m01", [128, NKC * RPC], BF16,
                                        isOutput=False)
    out = nc.declare_dram_parameter("out", [RPC, D], F32, isOutput=True)

    with tile.TileContext(nc) as tc:
        with (
            tc.tile_pool(name="res", bufs=1) as res,          # resident
            tc.tile_pool(name="stream2m", bufs=2) as stream2m,  # 2MB blocks
            tc.tile_pool(name="stream05", bufs=7) as stream05,  # 0.5MB blocks
            tc.tile_pool(name="small", bufs=3) as small,
            tc.tile_pool(name="probs", bufs=8) as probsp,
            tc.tile_pool(name="bcast", bufs=2) as bcastp,
            tc.tile_pool(name="dram", bufs=1, space="DRAM") as dramp,
            tc.tile_pool(name="psmm", bufs=5, space="PSUM") as psmm,
            tc.tile_pool(name="psacc", bufs=2, space="PSUM") as psacc,
            tc.tile_pool(name="pssum", bufs=1, space="PSUM") as pssum,
        ):
            # ---------------- resident tiles (DMAs staged per phase) -------
            # K-proj operands stream first so the first matmul fires ASAP;
            # the small constant tiles follow them in the queue.
            kmov = stream2m.tile([128, NIC, 512], BF16, tag="s2m")
            nc.sync.dma_start(
                out=kmov[:, 0:4, :],
                in_=kt[:, 0:4 * 512].rearrange("p (i m) -> p i m", i=4))
            wk0 = stream05.tile([128, NIC, 128], BF16, tag="s05")
            # split so the first matmul only waits on the first quarter
            nc.sync.dma_start(out=wk0[:, 0:4, :], in_=wk[0, :, 0:4 * 128]
                              .rearrange("p (i m) -> p i m", i=4))
            nc.sync.dma_start(out=wk0[:, 4:, :], in_=wk[0, :, 4 * 128:]
                              .rearrange("p (i m) -> p i m", i=NIC - 4))
            for icq in range(1, 4):
                nc.sync.dma_start(
                    out=kmov[:, 4 * icq:4 * icq + 4, :],
                    in_=kt[:, 4 * icq * 512:(4 * icq + 4) * 512].rearrange(
                        "p (i m) -> p i m", i=4))
            coskq_t = res.tile([128, 512], BF16)
            nc.sync.dma_start(out=coskq_t, in_=cosk[:, :])
            sinkq_t = res.tile([128, 512], BF16)
            nc.sync.dma_start(out=sinkq_t, in_=sink[:, :])
            pswap_t = res.tile([128, 128], BF16)
            nc.sync.dma_start(out=pswap_t, in_=pswap[:, :])
            ones_t = res.tile([128, 1], BF16)
            nc.vector.memset(ones_t, 1.0)
            if mode == "causal":
                mdiag_t = res.tile([128, 32], BF16)
                nc.sync.dma_start(out=mdiag_t, in_=mdiag[:, :])
            # Q-proj inputs enqueued early so their DMA descriptors sit
            # ahead of anything that waits on the collectives
            qts = res.tile([128, NIC, RPC], BF16)
            nc.sync.dma_start(out=qts, in_=qt[:, :].rearrange(
                "p (i m) -> p i m", i=NIC))
            cosq_t = res.tile([128, RPC], BF16)
            nc.sync.dma_start(out=cosq_t, in_=cosq[:, :])
            sinq_t = res.tile([128, RPC], BF16)
            nc.sync.dma_start(out=sinq_t, in_=sinq[:, :])

            qhs = res.tile([128, H, RPC], BF16)     # rope'd q, [dh, h, rows]
            # gathered K/V live in per-chunk tiles so chunk-A consumers
            # only depend on chunk-A unstage DMAs (hazard tracking on one
            # shared tile serializes attention behind the whole gather)
            khsA = res.tile([128, HKV, 4, 256], BF16)
            khsB = res.tile([128, HKV, 4, 256], BF16)
            vhsA = res.tile([128, 8, DKV], BF16)
            vhsB = res.tile([128, 8, DKV], BF16)

            def kv_aps(kc, hk):
                r, m = divmod(kc, 4)
                kt_, vt_ = (khsA, vhsA) if m < 2 else (khsB, vhsB)
                m = m % 2
                return (kt_[:, hk, r, m * 128:(m + 1) * 128],
                        vt_[:, 2 * r + m, hk * 128:(hk + 1) * 128])
            # outu_a shares qts's slot: qts is dead once phase A finishes.
            # split 12/4 so phase D's early matmuls (h<12) don't dep-chain
            # behind the last normalization batch (h>=12).
            outu_a = res.tile([128, 12, RPC], BF16, tag="qts")
            outu_b = res.tile([128, 4, RPC], BF16)

            def outu(h):
                return outu_a[:, h, :] if h < 12 else outu_b[:, h - 12, :]
            # normalization batches: the last batch (latency-exposed right
            # before the out-projection) covers only heads 14-15
            NB = [(0, 8), (8, 14), (14, 16)]
            sums_g = [res.tile([8, RPC], F32, name=f"sums{g}", tag=f"sums{g}")
                      for g in range(len(NB))]
            rec_g = [res.tile([8, RPC], F32, name=f"rec{g}", tag=f"rec{g}")
                     for g in range(len(NB))]
            sums_dram = dramp.tile([16, RPC], F32)
            rec_dram = dramp.tile([16, RPC], F32)
            khs_own = res.tile([128, HKV, 512], BF16)
            vhs_own = res.tile([128, 4, DKV], BF16)
            # 2-chunk staging (2 blocks each); K first, then V
            kv_cinA = dramp.tile([128, 2048], BF16)
            kv_coutA = dramp.tile([4, 128, 2048], BF16)
            kv_cinB = dramp.tile([128, 2048], BF16)
            kv_coutB = dramp.tile([4, 128, 2048], BF16)

            def mm_dedup(out, lhsT, rhs, **kw):
                """Matmul that reuses the PE's already-loaded stationary
                operand (the immediately preceding matmul used the same
                lhsT), skipping the redundant LDWEIGHTS."""
                inst = nc.tensor.matmul(out, lhsT, rhs, **kw)
                try:
                    inst.ins.ldweights = False
                except Exception:
                    pass
                return inst

            def rope(dst, x_bf, ps_pool, cos_ap, sin_ap, n):
                """dst = x*cos + pairswap(x)*sin  (signs baked into sin)."""
                y_ps = ps_pool.tile([128, 512], F32, tag="mm")
                # moving operand max 1024 bf16 per matmul
                assert n <= 512
                nc.tensor.matmul(y_ps[:, :n], pswap_t, x_bf, start=True,
                                 stop=True)
                t1 = small.tile([128, 512], BF16, tag="t1")
                nc.vector.tensor_mul(t1[:, :n], x_bf, cos_ap)
                t2 = small.tile([128, 512], BF16, tag="t2")
                nc.vector.tensor_mul(t2[:, :n], y_ps[:, :n], sin_ap)
                nc.vector.tensor_add(dst, t1[:, :n], t2[:, :n])

            # ------- Phase B: K/V proj for OWN 512-token quarter + RoPE -----
            # (first, so the chunked all-gather overlaps Q proj + attention)
            for hk in range(HKV):
                if hk == 0:
                    wk_all = wk0
                else:
                    wk_all = stream05.tile([128, NIC, 128], BF16, tag="s05")
                    nc.sync.dma_start(out=wk_all, in_=wk[hk].rearrange(
                        "p (i m) -> p i m", i=NIC))
                ps = psmm.tile([128, 512], F32, tag="mm")
                for ic in range(NIC):
                    nc.tensor.matmul(ps, wk_all[:, ic, :],
                                     kmov[:, ic, :],
                                     start=(ic == 0), stop=(ic == NIC - 1))
                xk = small.tile([128, 512], BF16, tag="xq")
                nc.scalar.copy(xk, ps)
                rope(khs_own[:, hk, :], xk, psmm, coskq_t, sinkq_t, 512)

            wvs = res.tile([128, NIC, DKV], BF16)
            nc.sync.dma_start(out=wvs, in_=wv[:, :].rearrange(
                "p (i n) -> p i n", i=NIC))
            for j in range(4):            # own 128-token blocks (V stationary)
                vmov = stream05.tile([128, NIC, 128], BF16, tag="s05")
                nc.sync.dma_start(out=vmov, in_=vt[j].rearrange(
                    "p (i m) -> p i m", i=NIC))
                ps = psmm.tile([128, 512], F32, tag="mm")
                for ic in range(NIC):
                    nc.tensor.matmul(ps, vmov[:, ic, :],
                                     wvs[:, ic, :],
                                     start=(ic == 0), stop=(ic == NIC - 1))
                nc.vector.tensor_copy(vhs_own[:, j, :], ps)
                if j % 2 == 0:
                    continue
                # stage + all-gather chunk: j==1 -> blocks {0,1} (A),
                # j==3 -> blocks {2,3} (B)
                blo, bhi = CHUNK_BLKS[j // 2]
                nb = bhi - blo
                kv_cin = kv_cinA if j == 1 else kv_cinB
                kv_cout = kv_coutA if j == 1 else kv_coutB
                ksz = HKV * 128 * nb
                nc.sync.dma_start(
                    out=kv_cin[:, 0:ksz].rearrange("p (h m) -> p h m", h=HKV),
                    in_=khs_own[:, :, 128 * blo:128 * bhi])
                nc.sync.dma_start(
                    out=kv_cin[:, ksz:].rearrange("p (v m) -> p v m", v=nb),
                    in_=vhs_own[:, blo:bhi, :])
                nc.gpsimd.collective_compute(
                    "AllGather", mybir.AluOpType.bypass,
                    replica_groups=[[0, 1, 2, 3], [4, 5, 6, 7]],
                    ins=[kv_cin[:, :]], outs=[kv_cout[:, :, :]])
                # NOTE: the unstage DMAs are enqueued AFTER the Q-proj
                # input loads (below) -- their descriptors wait on the CC
                # semaphore and would head-of-line-block every input
                # stream descriptor behind them in the DMA rings.

            def unstage(j):
                blo, bhi = CHUNK_BLKS[j // 2]
                nb = bhi - blo
                kv_cout = kv_coutA if j == 1 else kv_coutB
                ksz = HKV * 128 * nb
                kdst = khsA if j == 1 else khsB
                vdst = vhsA if j == 1 else vhsB
                for r in range(4):
                    nc.sync.dma_start(
                        out=kdst[:, :, r, :],
                        in_=kv_cout[r, :, 0:ksz].rearrange(
                            "p (h x) -> p h x", h=HKV))
                    nc.sync.dma_start(
                        out=vdst[:, 2 * r:2 * r + 2, :],
                        in_=kv_cout[r, :, ksz:].rearrange(
                            "p (v x) -> p v x", v=nb))

            # ---------------- Phase A: Q-proj + RoPE ----------------
            def qproj(oc):
                wq_all = stream05.tile([128, NIC, 128], BF16, tag="s05",
                                       name="wq_all")
                nc.sync.dma_start(out=wq_all, in_=wq[oc].rearrange(
                    "p (i m) -> p i m", i=NIC))
                ps = psmm.tile([128, 512], F32, tag="mm", name="ps_q")
                for ic in range(NIC):
                    nc.tensor.matmul(ps, wq_all[:, ic, :],
                                     qts[:, ic, :],
                                     start=(ic == 0), stop=(ic == NIC - 1))
                xq = small.tile([128, 512], BF16, tag="xq", name="xq")
                nc.scalar.copy(xq, ps)
                rope(qhs[:, oc, :], xq, psmm, cosq_t, sinq_t, RPC)

            for oc in range(12):
                qproj(oc)

            # ---------------- Phase C: attention per head ----------------
            if mode == "mask":
                m01s = res.tile([128, NKC, RPC], BF16)
                nc.sync.dma_start(out=m01s, in_=m01[:, :].rearrange(
                    "p (k m) -> p k m", k=NKC))
            # unstage the gathered K/V chunks (descriptors wait on the CC
            # semaphores -- enqueued after the bulk of the input streams
            # so they don't head-of-line-block the DMA rings)
            unstage(1)

            def normalize_batch(g):
                """reciprocal + broadcast + in-place normalize for the heads
                of batch g (their sums are already in sums_dram)."""
                a, bnd = NB[g]
                m = bnd - a
                nc.sync.dma_start(out=sums_g[g][:m, :],
                                  in_=sums_dram[a:bnd, :])
                nc.vector.reciprocal(rec_g[g][:m, :], sums_g[g][:m, :])
                nc.sync.dma_start(out=rec_dram[a:bnd, :], in_=rec_g[g][:m, :])
                for h in range(a, bnd):
                    recb = bcastp.tile([128, RPC], F32, tag="bc")
                    nc.sync.dma_start(
                        out=recb,
                        in_=rec_dram[h:h + 1, :].to_broadcast([128, RPC]))
                    nc.vector.tensor_mul(outu(h), outu(h), recb)

            LA = 2                # kc-step lookahead (software pipeline)
            # chunk-A partial PV sums (bf16) alias the dead wvs slot;
            # partial row-sums are tiny and get their own tile
            partA_o = res.tile([128, NIC, DKV], BF16, tag="wvs",
                               name="partA_o")
            # per-head partial row-sums: head h at partition 32*(h%4),
            # column slot h//4 (DVE writes must start 32-aligned)
            partA_s = res.tile([128, 4, RPC], F32, name="partA_s")

            def pas(h, c0=0, c1=RPC):
                p = 32 * (h % 4)
                return partA_s[p:p + 1, h // 4, c0:c1]
            # columns below BLO are never written by the B phase (causal:
            # every B-phase kc >= 2 has lo >= 64); they are copied from the
            # A partials instead of added
            BLO = 64 if mode == "causal" else 0

            def attn_pair(hp, kc_list, phase):
                h0, h1 = 2 * hp, 2 * hp + 1
                hk = h0 // G
                nkc = len(kc_list)
                ps_o0 = psacc.tile([128, 512], F32, tag="acc", name="ps_o0")
                ps_o1 = psacc.tile([128, 512], F32, tag="acc", name="ps_o1")
                # both heads' row-sums in one bank: h0 on partition 0,
                # h1 on partition 32 (column-group tiling)
                ps_s = pssum.tile([128, 512], F32, tag="sum", name="ps_s")
                pending = {}

                def issue_scores(i):
                    kc = kc_list[i]
                    n = n_of[kc]
                    lo = RPC - n          # suffix columns
                    kap, _ = kv_aps(kc, hk)
                    packed = 2 * n <= 512
                    if packed:
                        # one matmul computes both heads' scores: the moving
                        # operand is the 3D slice [128, 2 heads, n]
                        ps_sc = psmm.tile([128, 512], F32, tag="mm")
                        nc.tensor.matmul(ps_sc[:, :2 * n], kap,
                                         qhs[:, h0:h0 + 2, lo:],
                                         start=True, stop=True,
                                         skip_group_check=True)
                        probs = probsp.tile([128, 512], BF16, tag="pr")
                        nc.scalar.activation(
                            probs[:, :2 * n], ps_sc[:, :2 * n],
                            mybir.ActivationFunctionType.Exp, scale=SCALE)
                        pr0, pr1 = probs[:, :n], probs[:, n:2 * n]
                        d0, d1 = probs[:, 0:32], probs[:, n:n + 32]
                    else:
                        ps_a = psmm.tile([128, 512], F32, tag="mm")
                        nc.tensor.matmul(ps_a[:, :n], kap, qhs[:, h0, lo:],
                                         start=True, stop=True,
                                         skip_group_check=True)
                        ps_b = psmm.tile([128, 512], F32, tag="mm")
                        mm_dedup(ps_b[:, :n], kap, qhs[:, h1, lo:],
                                 start=True, stop=True,
                                 skip_group_check=True)
                        probs0 = probsp.tile([128, 512], BF16, tag="pr")
                        nc.scalar.activation(
                            probs0[:, :n], ps_a[:, :n],
                            mybir.ActivationFunctionType.Exp, scale=SCALE)
                        probs1 = probsp.tile([128, 512], BF16, tag="pr")
                        nc.scalar.activation(
                            probs1[:, :n], ps_b[:, :n],
                            mybir.ActivationFunctionType.Exp, scale=SCALE)
                        pr0, pr1 = probs0[:, :n], probs1[:, :n]
                        d0, d1 = probs0[:, 0:32], probs1[:, 0:32]
                    if mode == "causal":
                        # only the first 32 suffix columns (the diagonal
                        # 32-row q-block) are partially masked
                        nc.vector.tensor_mul(d0, d0, mdiag_t)
                        nc.vector.tensor_mul(d1, d1, mdiag_t)
                    elif mask_mul:
                        map_ = m01s[:, kc, lo:]
                        nc.vector.tensor_mul(pr0, pr0, map_)
                        nc.vector.tensor_mul(pr1, pr1, map_)
                    pending[i] = (pr0, pr1, kc, n, lo)

                for i in range(LA):
                    issue_scores(i)
                for idx in range(nkc):
                    if idx + LA < nkc:
                        issue_scores(idx + LA)
                    pr0, pr1, kc, n, lo = pending.pop(idx)
                    first = idx == 0
                    last = idx == nkc - 1
                    _, vap = kv_aps(kc, hk)
                    # the two row-sum matmuls target different column
                    # groups of the PE array and run concurrently
                    nc.tensor.matmul(ps_s[0:1, lo:], ones_t, pr0,
                                     start=first, stop=last,
                                     skip_group_check=True,
                                     tile_position=(0, 0))
                    nc.tensor.matmul(ps_s[32:33, lo:], ones_t, pr1,
                                     start=first, stop=last,
                                     skip_group_check=True,
                                     tile_position=(0, 32))
                    nc.tensor.matmul(ps_o0[:, lo:], vap, pr0,
                                     start=first, stop=last,
                                     skip_group_check=True)
                    mm_dedup(ps_o1[:, lo:], vap, pr1,
                             start=first, stop=last,
                             skip_group_check=True)
                for h, strip, ps_o in ((h0, 0, ps_o0), (h1, 32, ps_o1)):
                    if phase == "A":
                        # save chunk-A partials; B pass completes them
                        nc.vector.tensor_copy(partA_o[:, h, :], ps_o)
                        nc.vector.tensor_copy(pas(h),
                                              ps_s[strip:strip + 1, :])
                        continue
                    if phase == "F":
                        # single full pass (both chunks were already
                        # gathered when this pair started)
                        sm1 = small.tile([1, RPC], F32, tag="sm1", bufs=2)
                        nc.vector.tensor_copy(sm1, ps_s[strip:strip + 1, :])
                        nc.sync.dma_start(out=sums_dram[h:h + 1, :], in_=sm1)
                        nc.vector.tensor_copy(outu(h), ps_o)
                        continue
                    sm1 = small.tile([1, RPC], F32, tag="sm1", bufs=2)
                    if BLO:
                        nc.vector.tensor_copy(sm1[:, :BLO], pas(h, 0, BLO))
                        nc.vector.tensor_copy(
                            outu_a[:, h, :BLO] if h < 12
                            else outu_b[:, h - 12, :BLO],
                            partA_o[:, h, :BLO])
                    nc.vector.tensor_add(sm1[:, BLO:], pas(h, BLO, RPC),
                                         ps_s[strip:strip + 1, BLO:])
                    nc.sync.dma_start(out=sums_dram[h:h + 1, :], in_=sm1)
                    nc.vector.tensor_add(
                        outu_a[:, h, BLO:] if h < 12
                        else outu_b[:, h - 12, BLO:],
                        partA_o[:, h, BLO:], ps_o[:, BLO:])

            # the first two A pairs interleave with the tail of Q-proj so
            # ScalarE's exp stream starts as soon as chunk A lands
            attn_pair(0, KC_A, "A")
            attn_pair(1, KC_A, "A")
            for oc in range(12, 16):
                qproj(oc)
            unstage(3)
            for hp in range(2, 6):
                attn_pair(hp, KC_A, "A")
            # pairs 6-7 start after chunk B has landed: run them as
            # single full passes (no partial save/restore, and their
            # sums complete early enough that the last normalize
            # batches fire right after B-pair 5)
            attn_pair(6, KC_A + KC_B, "F")
            attn_pair(7, KC_A + KC_B, "F")
            for hp in range(6):
                attn_pair(hp, KC_B, "B")
                if hp == 3:
                    normalize_batch(0)
                elif hp == 5:
                    normalize_batch(1)
            normalize_batch(2)

            # ---------------- Phase D: out-projection ----------------
            for oc in range(4):
                wo_all = stream2m.tile([128, H, 512], BF16, tag="s2m")
                nc.sync.dma_start(out=wo_all, in_=wo[oc].rearrange(
                    "p (h m) -> p h m", h=H))
                for qc in range(4):
                    ps_f = psmm.tile([128, 512], F32, tag="mm")
                    for h in range(H):
                        lh = outu_a[:, h, qc * 128:(qc + 1) * 128] if h < 12 \
                            else outu_b[:, h - 12, qc * 128:(qc + 1) * 128]
                        nc.tensor.matmul(
                            ps_f, lh, wo_all[:, h, :],
                            start=(h == 0), stop=(h == H - 1))
                    fin = small.tile([128, 512], F32, tag="fin")
                    nc.vector.tensor_copy(fin, ps_f)
                    nc.sync.dma_start(
                        out=out[qc * 128:(qc + 1) * 128,
                                oc * 512:(oc + 1) * 512],
                        in_=fin)

    nc.compile()
    return nc


def _get_nc(mode: str):
    if mode not in _NC_CACHE:
        _NC_CACHE[mode] = _build(mode)
    return _NC_CACHE[mode]


def _core_rows(mode: str, r: int) -> np.ndarray:
    """Global (within-batch) q-row indices owned by quarter r, ascending.

    causal: 16 interleaved 32-row blocks {4j + r : j} -> exact suffix
    causality, identical shapes on every core.  other modes: contiguous.
    """
    if mode == "causal":
        return np.concatenate([np.arange(32 * (4 * j + r), 32 * (4 * j + r + 1))
                               for j in range(16)])
    return np.arange(r * RPC, (r + 1) * RPC)


def kernel(q, k, v, mask, freqs, W_q, W_k, W_v, W_o):
    q = np.asarray(q, dtype=np.float32)
    k = np.asarray(k, dtype=np.float32)
    v = np.asarray(v, dtype=np.float32)
    mask = np.asarray(mask, dtype=np.float32)
    freqs = np.asarray(freqs, dtype=np.float32)
    W_q = np.asarray(W_q, dtype=np.float32)
    W_k = np.asarray(W_k, dtype=np.float32)
    W_v = np.asarray(W_v, dtype=np.float32)
    W_o = np.asarray(W_o, dtype=np.float32)

    # ---- mask mode detection ----
    nz = mask != 0
    if nz.all():
        mode = "none"
    else:
        tril = np.tril(np.ones((S, S), dtype=bool))
        mode = "causal" if all(np.array_equal(nz[b], tril) for b in range(B)) \
            else "mask"

    # ---- shared host precomputation ----
    c_full = np.cos(freqs)                      # [S, 64]
    s_full = np.sin(freqs)
    sgn = np.tile(np.array([-1.0, 1.0], np.float32), DH // 2)  # [-,+,-,+...]
    cosk_h = np.repeat(c_full, 2, axis=1).T.astype(BF)          # [128, S]
    sink_h = (np.repeat(s_full, 2, axis=1) * sgn).T.astype(BF)

    psw = np.zeros((128, 128), np.float32)
    idx = np.arange(128)
    psw[idx, idx ^ 1] = 1.0
    psw = psw.astype(BF)

    # weight layouts
    # wq[oc, p, i*128+m] = W_q[oc*128+m, i*128+p]
    wq_h = np.ascontiguousarray(
        W_q.reshape(H, 128, NIC, 128).transpose(0, 3, 2, 1)
        .reshape(H, 128, D)).astype(BF)
    # wk[hk, p, i*128+m] = W_k[hk*128+m, i*128+p]
    wk_h = np.ascontiguousarray(
        W_k.reshape(HKV, 128, NIC, 128).transpose(0, 3, 2, 1)
        .reshape(HKV, 128, D)).astype(BF)
    # wv[p, i*512+n] = W_v[n, i*128+p]
    wv_h = np.ascontiguousarray(
        W_v.reshape(DKV, NIC, 128).transpose(2, 1, 0).reshape(128, NIC * DKV)
    ).astype(BF)
    # wo[oc, p, h*512+m] = W_o[oc*512+m, h*128+p]
    wo_h = np.ascontiguousarray(
        W_o.reshape(4, 512, H, 128).transpose(0, 3, 2, 1).reshape(4, 128, -1)
    ).astype(BF)

    # k/v: each core only gets its own 512-token quarter (gathered on device)
    # kt[p, i*512+t] = k[b, tq*512+t, i*128+p] for quarter tq
    kt_b = []   # [B][4] quarters
    vt_b = []
    for b in range(B):
        kt_b.append([np.ascontiguousarray(
            k[b, tq * 512:(tq + 1) * 512].reshape(512, NIC, 128)
            .transpose(2, 1, 0).reshape(128, NIC * 512)).astype(BF)
            for tq in range(4)])
        # vt[j, p, i*128+t] = v[b, tq*512 + j*128+t, i*128+p]
        vt_b.append([np.ascontiguousarray(
            v[b, tq * 512:(tq + 1) * 512].reshape(4, 128, NIC, 128)
            .transpose(0, 3, 2, 1).reshape(4, 128, NIC * 128)).astype(BF)
            for tq in range(4)])

    in_maps = []
    rows_all = []
    for c in range(NCORES):
        b, r = divmod(c, 4)
        rows = _core_rows(mode, r)
        rows_all.append((b, rows))
        # qt[p, i*512+t] = q[b, rows[t], i*128+p]
        qsl = q[b][rows]                       # [512, D]
        qt_h = np.ascontiguousarray(
            qsl.reshape(RPC, NIC, 128).transpose(2, 1, 0).reshape(128, -1)
        ).astype(BF)
        cq = np.repeat(c_full[rows], 2, axis=1).T.astype(BF)      # [128, 512]
        sq = (np.repeat(s_full[rows], 2, axis=1) * sgn).T.astype(BF)
        im = {
            "wq": wq_h, "qt": qt_h, "kt": kt_b[b][r], "vt": vt_b[b][r],
            "wk": wk_h, "wv": wv_h, "wo": wo_h,
            "cosq": cq, "sinq": sq,
            "cosk": np.ascontiguousarray(cosk_h[:, r * 512:(r + 1) * 512]),
            "sink": np.ascontiguousarray(sink_h[:, r * 512:(r + 1) * 512]),
            "pswap": psw,
        }
        if mode == "causal":
            # diagonal 32-col block mask: keep key p of tile kc for local
            # row i of block j=kc  <=>  p <= 32*r + i  (same for all kc)
            pp = np.arange(128)[:, None]
            ii = np.arange(32)[None, :]
            im["mdiag"] = (pp <= 32 * r + ii).astype(BF)
        elif mode == "mask":
            # m01[p, kc*512+m] = (mask[b, rows[m], kc*128+p] != 0)
            msl = nz[b][rows]                  # [512, S] bool
            m01_h = np.ascontiguousarray(
                msl.T.reshape(NKC, 128, RPC).transpose(1, 0, 2)
                .reshape(128, -1)).astype(BF)
            im["m01"] = m01_h
        in_maps.append(im)

    nc = _get_nc(mode)
    kwargs = {}
    if TRACE:
        kwargs["trace"] = True
        if TRACE_CORES:
            kwargs["trace_cores"] = list(TRACE_CORES)
    results = run_bass_kernel_spmd(nc, in_maps, core_ids=list(range(NCORES)),
                                   **kwargs)
    global LAST_RESULTS
    LAST_RESULTS = results

    full = np.empty((B, S, D), np.float32)
    for c in range(NCORES):
        b, rows = rows_all[c]
        full[b, rows] = results.results[c]["out"]
    return full


# revision 69
# speedup vs baseline: 1.0067x; 1.0067x over previous
"""Trainium2 Bass kernel for MultiHead GQA attention (B=2, S=2048, D=2048,
H=16 query heads, HKV=4 kv heads, DH=128, RoPE, mask, out-proj).

Sharding: token-parallel across 8 cores. Core c handles batch c//4 and 512
query rows of it. Each core projects K/V for its own 512-token quarter
(all 4 kv heads), the quarters are all-gathered in 4 pipelined 128-token
chunks, and the core runs attention + out-proj for its rows. Host
reassembles. All matmuls bf16 with fp32 PSUM accumulation.

Causal handling (exact, SPMD-uniform): core r of its batch owns the 16
interleaved 32-row q-blocks {4j + r : j=0..15} (ascending). For key tile
kc the q-blocks that attend to it are exactly the suffix of blocks with
position j >= kc, i.e. a contiguous column suffix of width n = 32*(16-kc)
-- identical on every core. Only the first 32 columns of each suffix (the
diagonal block) are partially masked; they get multiplied by a per-core
[128, 32] 0/1 tile. This computes 34 128x128-tile-equivalents per head
(the exact causal minimum for a 4-way row split) vs 40 for the previous
128-row-block scheme.

Attention is computed transposed: scoresT[keys, q] = khT.T @ qhT per
128-key tile, exp on ScalarE (scale folded in), probs bf16, then
outT[dh, q] += v_tile.T @ probsT, and row-sums via a ones-stationary
matmul. outT feeds the out-projection directly as stationary operand.

The K/V all-gather is split into 4 collectives, one per 128-token block
of each rank's quarter: chunk m delivers key tiles {4r + m : r=0..3}.
Attention iterates kc in the order [0,4,8,12, 1,5,9,13, ...] so the
first kc group only needs chunk 0 -- the remaining chunks stream in
behind attention/Q-proj compute instead of serializing in front of it.

Mask modes (host-detected, compile-time): none / causal / mask as before;
"mask" computes the full rectangle (n=512) and multiplies by the 0/1 mask.
"""

import math

import numpy as np
import ml_dtypes

import concourse.bass as bass
import concourse.mybir as mybir
import concourse.tile as tile
from concourse import bacc
from concourse.bass_utils import run_bass_kernel_spmd

F32 = mybir.dt.float32
BF16 = mybir.dt.bfloat16
BF = ml_dtypes.bfloat16

B, S, D = 2, 2048, 2048
H, G = 16, 4
HKV = H // G            # 4
DH = D // H             # 128
DKV = D // G            # 512 (kv projection width)
NCORES = 8
RPC = S // 4            # 512 rows per core
NIC = D // 128          # 16 contraction chunks
NKC = S // 128          # 16 key tiles
SCALE = 1.0 / math.sqrt(DH)
# attention kc order: the K/V all-gather is split in 2 chunks; chunk A
# carries 128-token blocks {0,1} of each rank's quarter (= key tiles
# {4r, 4r+1}), chunk B blocks {2,3}.  Attention runs in two phases: an
# A pass over all head pairs (chunk-A tiles only, partial sums saved to
# SBUF), then a B pass -- so the A pass needs only chunk A, and chunk B
# arrives long before the B pass starts.
CHUNK_BLKS = [(0, 2), (2, 4)]        # [lo, hi) own-token-block range per chunk
KC_A = [4 * b + m for b in range(4) for m in range(2)]
KC_B = [4 * b + m for b in range(4) for m in range(2, 4)]

_NC_CACHE: dict = {}

# set by callers (e.g. test.py) to capture a profile; results of the last run
TRACE = False
TRACE_CORES = None          # e.g. [0] or list(range(8))
LAST_RESULTS = None


def _n_list(mode: str) -> list[int]:
    """Moving-operand width (in q columns, suffix of the 512) per key tile."""
    if mode == "causal":
        return [32 * (16 - kc) for kc in range(NKC)]
    return [512] * NKC


def _build(mode: str):
    mask_mul = mode != "none"
    n_of = _n_list(mode)

    nc = bacc.Bacc("TRN2", target_bir_lowering=False, debug=False,
                   num_devices=NCORES)

    # ---- I/O (host-prepared layouts; all contiguous-DMA friendly) ----
    wq = nc.declare_dram_parameter("wq", [NIC, 128, D], BF16, isOutput=False)
    qt = nc.declare_dram_parameter("qt", [128, NIC * RPC], BF16, isOutput=False)
    # k/v: only this core's 512-token quarter (projected here, all-gathered)
    kt = nc.declare_dram_parameter("kt", [128, NIC * 512], BF16, isOutput=False)
    vt = nc.declare_dram_parameter("vt", [4, 128, NIC * 128], BF16, isOutput=False)
    wk = nc.declare_dram_parameter("wk", [HKV, 128, NIC * 128], BF16, isOutput=False)
    wv = nc.declare_dram_parameter("wv", [128, NIC * DKV], BF16, isOutput=False)
    wo = nc.declare_dram_parameter("wo", [4, 128, H * 512], BF16, isOutput=False)
    cosq = nc.declare_dram_parameter("cosq", [128, RPC], BF16, isOutput=False)
    sinq = nc.declare_dram_parameter("sinq", [128, RPC], BF16, isOutput=False)
    # cos/sin for this core's own k-token quarter
    cosk = nc.declare_dram_parameter("cosk", [128, 512], BF16, isOutput=False)
    sink = nc.declare_dram_parameter("sink", [128, 512], BF16, isOutput=False)
    pswap = nc.declare_dram_parameter("pswap", [128, 128], BF16, isOutput=False)
    if mode == "causal":
        mdiag = nc.declare_dram_parameter("mdiag", [128, 32], BF16,
                                          isOutput=False)
    if mode == "mask":
        m01 = nc.declare_dram_parameter("m01", [128, NKC * RPC], BF16,
                                        isOutput=False)
    out = nc.declare_dram_parameter("out", [RPC, D], F32, isOutput=True)

    with tile.TileContext(nc) as tc:
        with (
            tc.tile_pool(name="res", bufs=1) as res,          # resident
            tc.tile_pool(name="stream2m", bufs=2) as stream2m,  # 2MB blocks
            tc.tile_pool(name="stream05", bufs=7) as stream05,  # 0.5MB blocks
            tc.tile_pool(name="small", bufs=3) as small,
            tc.tile_pool(name="probs", bufs=8) as probsp,
            tc.tile_pool(name="bcast", bufs=2) as bcastp,
            tc.tile_pool(name="dram", bufs=1, space="DRAM") as dramp,
            tc.tile_pool(name="psmm", bufs=5, space="PSUM") as psmm,
            tc.tile_pool(name="psacc", bufs=2, space="PSUM") as psacc,
            tc.tile_pool(name="pssum", bufs=1, space="PSUM") as pssum,
        ):
            # ---------------- resident tiles (DMAs staged per phase) -------
            # K-proj operands stream first so the first matmul fires ASAP;
            # the small constant tiles follow them in the queue.
            kmov = stream2m.tile([128, NIC, 512], BF16, tag="s2m")
            nc.sync.dma_start(
                out=kmov[:, 0:4, :],
                in_=kt[:, 0:4 * 512].rearrange("p (i m) -> p i m", i=4))
            wk0 = stream05.tile([128, NIC, 128], BF16, tag="s05")
            # split so the first matmul only waits on the first quarter
            nc.sync.dma_start(out=wk0[:, 0:4, :], in_=wk[0, :, 0:4 * 128]
                              .rearrange("p (i m) -> p i m", i=4))
            nc.sync.dma_start(out=wk0[:, 4:, :], in_=wk[0, :, 4 * 128:]
                              .rearrange("p (i m) -> p i m", i=NIC - 4))
            for icq in range(1, 4):
                nc.sync.dma_start(
                    out=kmov[:, 4 * icq:4 * icq + 4, :],
                    in_=kt[:, 4 * icq * 512:(4 * icq + 4) * 512].rearrange(
                        "p (i m) -> p i m", i=4))
            coskq_t = res.tile([128, 512], BF16)
            nc.sync.dma_start(out=coskq_t, in_=cosk[:, :])
            sinkq_t = res.tile([128, 512], BF16)
            nc.sync.dma_start(out=sinkq_t, in_=sink[:, :])
            pswap_t = res.tile([128, 128], BF16)
            nc.sync.dma_start(out=pswap_t, in_=pswap[:, :])
            ones_t = res.tile([128, 1], BF16)
            nc.vector.memset(ones_t, 1.0)
            if mode == "causal":
                mdiag_t = res.tile([128, 32], BF16)
                nc.sync.dma_start(out=mdiag_t, in_=mdiag[:, :])
            # Q-proj inputs enqueued early so their DMA descriptors sit
            # ahead of anything that waits on the collectives
            qts = res.tile([128, NIC, RPC], BF16)
            nc.sync.dma_start(out=qts, in_=qt[:, :].rearrange(
                "p (i m) -> p i m", i=NIC))
            cosq_t = res.tile([128, RPC], BF16)
            nc.sync.dma_start(out=cosq_t, in_=cosq[:, :])
            sinq_t = res.tile([128, RPC], BF16)
            nc.sync.dma_start(out=sinq_t, in_=sinq[:, :])

            qhs = res.tile([128, H, RPC], BF16)     # rope'd q, [dh, h, rows]
            # gathered K/V live in per-chunk tiles so chunk-A consumers
            # only depend on chunk-A unstage DMAs (hazard tracking on one
            # shared tile serializes attention behind the whole gather)
            khsA = res.tile([128, HKV, 4, 256], BF16)
            khsB = res.tile([128, HKV, 4, 256], BF16)
            vhsA = res.tile([128, 8, DKV], BF16)
            vhsB = res.tile([128, 8, DKV], BF16)

            def kv_aps(kc, hk):
                r, m = divmod(kc, 4)
                kt_, vt_ = (khsA, vhsA) if m < 2 else (khsB, vhsB)
                m = m % 2
                return (kt_[:, hk, r, m * 128:(m + 1) * 128],
                        vt_[:, 2 * r + m, hk * 128:(hk + 1) * 128])
            # outu_a shares qts's slot: qts is dead once phase A finishes.
            # split 12/4 so phase D's early matmuls (h<12) don't dep-chain
            # behind the last normalization batch (h>=12).
            outu_a = res.tile([128, 12, RPC], BF16, tag="qts")
            outu_b = res.tile([128, 4, RPC], BF16)

            def outu(h):
                return outu_a[:, h, :] if h < 12 else outu_b[:, h - 12, :]
            # normalization batches: the last batch (latency-exposed right
            # before the out-projection) covers only heads 14-15
            NB = [(0, 8), (8, 14), (14, 16)]
            sums_g = [res.tile([8, RPC], F32, name=f"sums{g}", tag=f"sums{g}")
                      for g in range(len(NB))]
            rec_g = [res.tile([8, RPC], F32, name=f"rec{g}", tag=f"rec{g}")
                     for g in range(len(NB))]
            sums_dram = dramp.tile([16, RPC], F32)
            rec_dram = dramp.tile([16, RPC], F32)
            khs_own = res.tile([128, HKV, 512], BF16)
            vhs_own = res.tile([128, 4, DKV], BF16)
            # 2-chunk staging (2 blocks each); K first, then V
            kv_cinA = dramp.tile([128, 2048], BF16)
            kv_coutA = dramp.tile([4, 128, 2048], BF16)
            kv_cinB = dramp.tile([128, 2048], BF16)
            kv_coutB = dramp.tile([4, 128, 2048], BF16)

            def mm_dedup(out, lhsT, rhs, **kw):
                """Matmul that reuses the PE's already-loaded stationary
                operand (the immediately preceding matmul used the same
                lhsT), skipping the redundant LDWEIGHTS."""
                inst = nc.tensor.matmul(out, lhsT, rhs, **kw)
                try:
                    inst.ins.ldweights = False
                except Exception:
                    pass
                return inst

            def rope(dst, x_bf, ps_pool, cos_ap, sin_ap, n):
                """dst = x*cos + pairswap(x)*sin  (signs baked into sin)."""
                y_ps = ps_pool.tile([128, 512], F32, tag="mm")
                # moving operand max 1024 bf16 per matmul
                assert n <= 512
                nc.tensor.matmul(y_ps[:, :n], pswap_t, x_bf, start=True,
                                 stop=True)
                t1 = small.tile([128, 512], BF16, tag="t1")
                nc.vector.tensor_mul(t1[:, :n], x_bf, cos_ap)
                t2 = small.tile([128, 512], BF16, tag="t2")
                nc.vector.tensor_mul(t2[:, :n], y_ps[:, :n], sin_ap)
                nc.vector.tensor_add(dst, t1[:, :n], t2[:, :n])

            # ------- Phase B: K/V proj for OWN 512-token quarter + RoPE -----
            # (first, so the chunked all-gather overlaps Q proj + attention)
            for hk in range(HKV):
                if hk == 0:
                    wk_all = wk0
                else:
                    wk_all = stream05.tile([128, NIC, 128], BF16, tag="s05")
                    nc.sync.dma_start(out=wk_all, in_=wk[hk].rearrange(
                        "p (i m) -> p i m", i=NIC))
                ps = psmm.tile([128, 512], F32, tag="mm")
                for ic in range(NIC):
                    nc.tensor.matmul(ps, wk_all[:, ic, :],
                                     kmov[:, ic, :],
                                     start=(ic == 0), stop=(ic == NIC - 1))
                xk = small.tile([128, 512], BF16, tag="xq")
                nc.scalar.copy(xk, ps)
                rope(khs_own[:, hk, :], xk, psmm, coskq_t, sinkq_t, 512)

            wvs = res.tile([128, NIC, DKV], BF16)
            nc.sync.dma_start(out=wvs, in_=wv[:, :].rearrange(
                "p (i n) -> p i n", i=NIC))
            for j in range(4):            # own 128-token blocks (V stationary)
                vmov = stream05.tile([128, NIC, 128], BF16, tag="s05")
                nc.sync.dma_start(out=vmov, in_=vt[j].rearrange(
                    "p (i m) -> p i m", i=NIC))
                ps = psmm.tile([128, 512], F32, tag="mm")
                for ic in range(NIC):
                    nc.tensor.matmul(ps, vmov[:, ic, :],
                                     wvs[:, ic, :],
                                     start=(ic == 0), stop=(ic == NIC - 1))
                nc.vector.tensor_copy(vhs_own[:, j, :], ps)
                if j % 2 == 0:
                    continue
                # stage + all-gather chunk: j==1 -> blocks {0,1} (A),
                # j==3 -> blocks {2,3} (B)
                blo, bhi = CHUNK_BLKS[j // 2]
                nb = bhi - blo
                kv_cin = kv_cinA if j == 1 else kv_cinB
                kv_cout = kv_coutA if j == 1 else kv_coutB
                ksz = HKV * 128 * nb
                nc.sync.dma_start(
                    out=kv_cin[:, 0:ksz].rearrange("p (h m) -> p h m", h=HKV),
                    in_=khs_own[:, :, 128 * blo:128 * bhi])
                nc.sync.dma_start(
                    out=kv_cin[:, ksz:].rearrange("p (v m) -> p v m", v=nb),
                    in_=vhs_own[:, blo:bhi, :])
                nc.gpsimd.collective_compute(
                    "AllGather", mybir.AluOpType.bypass,
                    replica_groups=[[0, 1, 2, 3], [4, 5, 6, 7]],
                    ins=[kv_cin[:, :]], outs=[kv_cout[:, :, :]])
                # NOTE: the unstage DMAs are enqueued AFTER the Q-proj
                # input loads (below) -- their descriptors wait on the CC
                # semaphore and would head-of-line-block every input
                # stream descriptor behind them in the DMA rings.

            def unstage(j):
                blo, bhi = CHUNK_BLKS[j // 2]
                nb = bhi - blo
                kv_cout = kv_coutA if j == 1 else kv_coutB
                ksz = HKV * 128 * nb
                kdst = khsA if j == 1 else khsB
                vdst = vhsA if j == 1 else vhsB
                for r in range(4):
                    nc.sync.dma_start(
                        out=kdst[:, :, r, :],
                        in_=kv_cout[r, :, 0:ksz].rearrange(
                            "p (h x) -> p h x", h=HKV))
                    nc.sync.dma_start(
                        out=vdst[:, 2 * r:2 * r + 2, :],
                        in_=kv_cout[r, :, ksz:].rearrange(
                            "p (v x) -> p v x", v=nb))

            # ---------------- Phase A: Q-proj + RoPE ----------------
            def qproj(oc):
                wq_all = stream05.tile([128, NIC, 128], BF16, tag="s05",
                                       name="wq_all")
                nc.sync.dma_start(out=wq_all, in_=wq[oc].rearrange(
                    "p (i m) -> p i m", i=NIC))
                ps = psmm.tile([128, 512], F32, tag="mm", name="ps_q")
                for ic in range(NIC):
                    nc.tensor.matmul(ps, wq_all[:, ic, :],
                                     qts[:, ic, :],
                                     start=(ic == 0), stop=(ic == NIC - 1))
                xq = small.tile([128, 512], BF16, tag="xq", name="xq")
                nc.scalar.copy(xq, ps)
                rope(qhs[:, oc, :], xq, psmm, cosq_t, sinq_t, RPC)

            for oc in range(12):
                qproj(oc)

            # ---------------- Phase C: attention per head ----------------
            if mode == "mask":
                m01s = res.tile([128, NKC, RPC], BF16)
                nc.sync.dma_start(out=m01s, in_=m01[:, :].rearrange(
                    "p (k m) -> p k m", k=NKC))
            # unstage the gathered K/V chunks (descriptors wait on the CC
            # semaphores -- enqueued after the bulk of the input streams
            # so they don't head-of-line-block the DMA rings)
            unstage(1)

            def normalize_batch(g):
                """reciprocal + broadcast + in-place normalize for the heads
                of batch g (their sums are already in sums_dram)."""
                a, bnd = NB[g]
                m = bnd - a
                nc.sync.dma_start(out=sums_g[g][:m, :],
                                  in_=sums_dram[a:bnd, :])
                nc.vector.reciprocal(rec_g[g][:m, :], sums_g[g][:m, :])
                nc.sync.dma_start(out=rec_dram[a:bnd, :], in_=rec_g[g][:m, :])
                for h in range(a, bnd):
                    recb = bcastp.tile([128, RPC], F32, tag="bc")
                    nc.sync.dma_start(
                        out=recb,
                        in_=rec_dram[h:h + 1, :].to_broadcast([128, RPC]))
                    nc.vector.tensor_mul(outu(h), outu(h), recb)

            LA = 2                # kc-step lookahead (software pipeline)
            # chunk-A partial PV sums (bf16) alias the dead wvs slot;
            # partial row-sums are tiny and get their own tile
            partA_o = res.tile([128, NIC, DKV], BF16, tag="wvs",
                               name="partA_o")
            # per-head partial row-sums: head h at partition 32*(h%4),
            # column slot h//4 (DVE writes must start 32-aligned)
            partA_s = res.tile([128, 4, RPC], F32, name="partA_s")

            def pas(h, c0=0, c1=RPC):
                p = 32 * (h % 4)
                return partA_s[p:p + 1, h // 4, c0:c1]
            # columns below BLO are never written by the B phase (causal:
            # every B-phase kc >= 2 has lo >= 64); they are copied from the
            # A partials instead of added
            BLO = 64 if mode == "causal" else 0

            def attn_pair(hp, kc_list, phase):
                h0, h1 = 2 * hp, 2 * hp + 1
                hk = h0 // G
                nkc = len(kc_list)
                ps_o0 = psacc.tile([128, 512], F32, tag="acc", name="ps_o0")
                ps_o1 = psacc.tile([128, 512], F32, tag="acc", name="ps_o1")
                # both heads' row-sums in one bank: h0 on partition 0,
                # h1 on partition 32 (column-group tiling)
                ps_s = pssum.tile([128, 512], F32, tag="sum", name="ps_s")
                pending = {}

                def issue_scores(i):
                    kc = kc_list[i]
                    n = n_of[kc]
                    lo = RPC - n          # suffix columns
                    kap, _ = kv_aps(kc, hk)
                    packed = 2 * n <= 512
                    if packed:
                        # one matmul computes both heads' scores: the moving
                        # operand is the 3D slice [128, 2 heads, n]
                        ps_sc = psmm.tile([128, 512], F32, tag="mm")
                        nc.tensor.matmul(ps_sc[:, :2 * n], kap,
                                         qhs[:, h0:h0 + 2, lo:],
                                         start=True, stop=True,
                                         skip_group_check=True)
                        probs = probsp.tile([128, 512], BF16, tag="pr")
                        nc.scalar.activation(
                            probs[:, :2 * n], ps_sc[:, :2 * n],
                            mybir.ActivationFunctionType.Exp, scale=SCALE)
                        pr0, pr1 = probs[:, :n], probs[:, n:2 * n]
                        d0, d1 = probs[:, 0:32], probs[:, n:n + 32]
                    else:
                        ps_a = psmm.tile([128, 512], F32, tag="mm")
                        nc.tensor.matmul(ps_a[:, :n], kap, qhs[:, h0, lo:],
                                         start=True, stop=True,
                                         skip_group_check=True)
                        ps_b = psmm.tile([128, 512], F32, tag="mm")
                        mm_dedup(ps_b[:, :n], kap, qhs[:, h1, lo:],
                                 start=True, stop=True,
                                 skip_group_check=True)
                        probs0 = probsp.tile([128, 512], BF16, tag="pr")
                        nc.scalar.activation(
                            probs0[:, :n], ps_a[:, :n],
                            mybir.ActivationFunctionType.Exp, scale=SCALE)
                        probs1 = probsp.tile([128, 512], BF16, tag="pr")
                        nc.scalar.activation(
                            probs1[:, :n], ps_b[:, :n],
                            mybir.ActivationFunctionType.Exp, scale=SCALE)
                        pr0, pr1 = probs0[:, :n], probs1[:, :n]
                        d0, d1 = probs0[:, 0:32], probs1[:, 0:32]
                    if mode == "causal":
                        # only the first 32 suffix columns (the diagonal
                        # 32-row q-block) are partially masked
                        nc.vector.tensor_mul(d0, d0, mdiag_t)
                        nc.vector.tensor_mul(d1, d1, mdiag_t)
                    elif mask_mul:
                        map_ = m01s[:, kc, lo:]
                        nc.vector.tensor_mul(pr0, pr0, map_)
                        nc.vector.tensor_mul(pr1, pr1, map_)
                    pending[i] = (pr0, pr1, kc, n, lo)

                for i in range(LA):
                    issue_scores(i)
                for idx in range(nkc):
                    if idx + LA < nkc:
                        issue_scores(idx + LA)
                    pr0, pr1, kc, n, lo = pending.pop(idx)
                    first = idx == 0
                    last = idx == nkc - 1
                    _, vap = kv_aps(kc, hk)
                    # the two row-sum matmuls target different column
                    # groups of the PE array and run concurrently
                    nc.tensor.matmul(ps_s[0:1, lo:], ones_t, pr0,
                                     start=first, stop=last,
                                     skip_group_check=True,
                                     tile_position=(0, 0))
                    nc.tensor.matmul(ps_s[32:33, lo:], ones_t, pr1,
                                     start=first, stop=last,
                                     skip_group_check=True,
                                     tile_position=(0, 32))
                    nc.tensor.matmul(ps_o0[:, lo:], vap, pr0,
                                     start=first, stop=last,
                                     skip_group_check=True)
                    mm_dedup(ps_o1[:, lo:], vap, pr1,
                             start=first, stop=last,
                             skip_group_check=True)
                for h, strip, ps_o in ((h0, 0, ps_o0), (h1, 32, ps_o1)):
                    if phase == "A":
                        # save chunk-A partials; B pass completes them
                        nc.vector.tensor_copy(partA_o[:, h, :], ps_o)
                        nc.vector.tensor_copy(pas(h),
                                              ps_s[strip:strip + 1, :])
                        continue
                    if phase == "F":
                        # single full pass (both chunks were already
                        # gathered when this pair started)
                        sm1 = small.tile([1, RPC], F32, tag="sm1", bufs=2)
                        nc.vector.tensor_copy(sm1, ps_s[strip:strip + 1, :])
                        nc.sync.dma_start(out=sums_dram[h:h + 1, :], in_=sm1)
                        nc.vector.tensor_copy(outu(h), ps_o)
                        continue
                    sm1 = small.tile([1, RPC], F32, tag="sm1", bufs=2)
                    if BLO:
                        nc.vector.tensor_copy(sm1[:, :BLO], pas(h, 0, BLO))
                        nc.vector.tensor_copy(
                            outu_a[:, h, :BLO] if h < 12
                            else outu_b[:, h - 12, :BLO],
                            partA_o[:, h, :BLO])
                    nc.vector.tensor_add(sm1[:, BLO:], pas(h, BLO, RPC),
                                         ps_s[strip:strip + 1, BLO:])
                    nc.sync.dma_start(out=sums_dram[h:h + 1, :], in_=sm1)
                    nc.vector.tensor_add(
                        outu_a[:, h, BLO:] if h < 12
                        else outu_b[:, h - 12, BLO:],
                        partA_o[:, h, BLO:], ps_o[:, BLO:])

            # the first two A pairs interleave with the tail of Q-proj so
            # ScalarE's exp stream starts as soon as chunk A lands
            attn_pair(0, KC_A, "A")
            attn_pair(1, KC_A, "A")
            for oc in range(12, 16):
                qproj(oc)
            unstage(3)
            for hp in range(2, H // 2):
                attn_pair(hp, KC_A, "A")
            for hp in range(H // 2):
                attn_pair(hp, KC_B, "B")
                if hp == 3:
                    normalize_batch(0)
                elif hp == 6:
                    normalize_batch(1)
            normalize_batch(2)

            # ---------------- Phase D: out-projection ----------------
            for oc in range(4):
                wo_all = stream2m.tile([128, H, 512], BF16, tag="s2m")
                nc.sync.dma_start(out=wo_all, in_=wo[oc].rearrange(
                    "p (h m) -> p h m", h=H))
                for qc in range(4):
                    ps_f = psmm.tile([128, 512], F32, tag="mm")
                    for h in range(H):
                        lh = outu_a[:, h, qc * 128:(qc + 1) * 128] if h < 12 \
                            else outu_b[:, h - 12, qc * 128:(qc + 1) * 128]
                        nc.tensor.matmul(
                            ps_f, lh, wo_all[:, h, :],
                            start=(h == 0), stop=(h == H - 1))
                    fin = small.tile([128, 512], F32, tag="fin")
                    nc.vector.tensor_copy(fin, ps_f)
                    nc.sync.dma_start(
                        out=out[qc * 128:(qc + 1) * 128,
                                oc * 512:(oc + 1) * 512],
                        in_=fin)

    nc.compile()
    return nc


def _get_nc(mode: str):
    if mode not in _NC_CACHE:
        _NC_CACHE[mode] = _build(mode)
    return _NC_CACHE[mode]


def _core_rows(mode: str, r: int) -> np.ndarray:
    """Global (within-batch) q-row indices owned by quarter r, ascending.

    causal: 16 interleaved 32-row blocks {4j + r : j} -> exact suffix
    causality, identical shapes on every core.  other modes: contiguous.
    """
    if mode == "causal":
        return np.concatenate([np.arange(32 * (4 * j + r), 32 * (4 * j + r + 1))
                               for j in range(16)])
    return np.arange(r * RPC, (r + 1) * RPC)


def kernel(q, k, v, mask, freqs, W_q, W_k, W_v, W_o):
    q = np.asarray(q, dtype=np.float32)
    k = np.asarray(k, dtype=np.float32)
    v = np.asarray(v, dtype=np.float32)
    mask = np.asarray(mask, dtype=np.float32)
    freqs = np.asarray(freqs, dtype=np.float32)
    W_q = np.asarray(W_q, dtype=np.float32)
    W_k = np.asarray(W_k, dtype=np.float32)
    W_v = np.asarray(W_v, dtype=np.float32)
    W_o = np.asarray(W_o, dtype=np.float32)

    # ---- mask mode detection ----
    nz = mask != 0
    if nz.all():
        mode = "none"
    else:
        tril = np.tril(np.ones((S, S), dtype=bool))
        mode = "causal" if all(np.array_equal(nz[b], tril) for b in range(B)) \
            else "mask"

    # ---- shared host precomputation ----
    c_full = np.cos(freqs)                      # [S, 64]
    s_full = np.sin(freqs)
    sgn = np.tile(np.array([-1.0, 1.0], np.float32), DH // 2)  # [-,+,-,+...]
    cosk_h = np.repeat(c_full, 2, axis=1).T.astype(BF)          # [128, S]
    sink_h = (np.repeat(s_full, 2, axis=1) * sgn).T.astype(BF)

    psw = np.zeros((128, 128), np.float32)
    idx = np.arange(128)
    psw[idx, idx ^ 1] = 1.0
    psw = psw.astype(BF)

    # weight layouts
    # wq[oc, p, i*128+m] = W_q[oc*128+m, i*128+p]
    wq_h = np.ascontiguousarray(
        W_q.reshape(H, 128, NIC, 128).transpose(0, 3, 2, 1)
        .reshape(H, 128, D)).astype(BF)
    # wk[hk, p, i*128+m] = W_k[hk*128+m, i*128+p]
    wk_h = np.ascontiguousarray(
        W_k.reshape(HKV, 128, NIC, 128).transpose(0, 3, 2, 1)
        .reshape(HKV, 128, D)).astype(BF)
    # wv[p, i*512+n] = W_v[n, i*128+p]
    wv_h = np.ascontiguousarray(
        W_v.reshape(DKV, NIC, 128).transpose(2, 1, 0).reshape(128, NIC * DKV)
    ).astype(BF)
    # wo[oc, p, h*512+m] = W_o[oc*512+m, h*128+p]
    wo_h = np.ascontiguousarray(
        W_o.reshape(4, 512, H, 128).transpose(0, 3, 2, 1).reshape(4, 128, -1)
    ).astype(BF)

    # k/v: each core only gets its own 512-token quarter (gathered on device)
    # kt[p, i*512+t] = k[b, tq*512+t, i*128+p] for quarter tq
    kt_b = []   # [B][4] quarters
    vt_b = []
    for b in range(B):
        kt_b.append([np.ascontiguousarray(
            k[b, tq * 512:(tq + 1) * 512].reshape(512, NIC, 128)
            .transpose(2, 1, 0).reshape(128, NIC * 512)).astype(BF)
            for tq in range(4)])
        # vt[j, p, i*128+t] = v[b, tq*512 + j*128+t, i*128+p]
        vt_b.append([np.ascontiguousarray(
            v[b, tq * 512:(tq + 1) * 512].reshape(4, 128, NIC, 128)
            .transpose(0, 3, 2, 1).reshape(4, 128, NIC * 128)).astype(BF)
            for tq in range(4)])

    in_maps = []
    rows_all = []
    for c in range(NCORES):
        b, r = divmod(c, 4)
        rows = _core_rows(mode, r)
        rows_all.append((b, rows))
        # qt[p, i*512+t] = q[b, rows[t], i*128+p]
        qsl = q[b][rows]                       # [512, D]
        qt_h = np.ascontiguousarray(
            qsl.reshape(RPC, NIC, 128).transpose(2, 1, 0).reshape(128, -1)
        ).astype(BF)
        cq = np.repeat(c_full[rows], 2, axis=1).T.astype(BF)      # [128, 512]
        sq = (np.repeat(s_full[rows], 2, axis=1) * sgn).T.astype(BF)
        im = {
            "wq": wq_h, "qt": qt_h, "kt": kt_b[b][r], "vt": vt_b[b][r],
            "wk": wk_h, "wv": wv_h, "wo": wo_h,
            "cosq": cq, "sinq": sq,
            "cosk": np.ascontiguousarray(cosk_h[:, r * 512:(r + 1) * 512]),
            "sink": np.ascontiguousarray(sink_h[:, r * 512:(r + 1) * 512]),
            "pswap": psw,
        }
        if mode == "causal":
            # diagonal 32-col block mask: keep key p of tile kc for local
            # row i of block j=kc  <=>  p <= 32*r + i  (same for all kc)
            pp = np.arange(128)[:, None]
            ii = np.arange(32)[None, :]
            im["mdiag"] = (pp <= 32 * r + ii).astype(BF)
        elif mode == "mask":
            # m01[p, kc*512+m] = (mask[b, rows[m], kc*128+p] != 0)
            msl = nz[b][rows]                  # [512, S] bool
            m01_h = np.ascontiguousarray(
                msl.T.reshape(NKC, 128, RPC).transpose(1, 0, 2)
                .reshape(128, -1)).astype(BF)
            im["m01"] = m01_h
        in_maps.append(im)

    nc = _get_nc(mode)
    kwargs = {}
    if TRACE:
        kwargs["trace"] = True
        if TRACE_CORES:
            kwargs["trace_cores"] = list(TRACE_CORES)
    results = run_bass_kernel_spmd(nc, in_maps, core_ids=list(range(NCORES)),
                                   **kwargs)
    global LAST_RESULTS
    LAST_RESULTS = results

    full = np.empty((B, S, D), np.float32)
    for c in range(NCORES):
        b, rows = rows_all[c]
        full[b, rows] = results.results[c]["out"]
    return full


# revision 71
# speedup vs baseline: 1.0294x; 1.0225x over previous
"""Trainium2 Bass kernel for MultiHead GQA attention (B=2, S=2048, D=2048,
H=16 query heads, HKV=4 kv heads, DH=128, RoPE, mask, out-proj).

Sharding: token-parallel across 8 cores. Core c handles batch c//4 and 512
query rows of it. Each core projects K/V for its own 512-token quarter
(all 4 kv heads), the quarters are all-gathered in 4 pipelined 128-token
chunks, and the core runs attention + out-proj for its rows. Host
reassembles. All matmuls bf16 with fp32 PSUM accumulation.

Causal handling (exact, SPMD-uniform): core r of its batch owns the 16
interleaved 32-row q-blocks {4j + r : j=0..15} (ascending). For key tile
kc the q-blocks that attend to it are exactly the suffix of blocks with
position j >= kc, i.e. a contiguous column suffix of width n = 32*(16-kc)
-- identical on every core. Only the first 32 columns of each suffix (the
diagonal block) are partially masked; they get multiplied by a per-core
[128, 32] 0/1 tile. This computes 34 128x128-tile-equivalents per head
(the exact causal minimum for a 4-way row split) vs 40 for the previous
128-row-block scheme.

Attention is computed transposed: scoresT[keys, q] = khT.T @ qhT per
128-key tile, exp on ScalarE (scale folded in), probs bf16, then
outT[dh, q] += v_tile.T @ probsT, and row-sums via a ones-stationary
matmul. outT feeds the out-projection directly as stationary operand.

The K/V all-gather is split into 4 collectives, one per 128-token block
of each rank's quarter: chunk m delivers key tiles {4r + m : r=0..3}.
Attention iterates kc in the order [0,4,8,12, 1,5,9,13, ...] so the
first kc group only needs chunk 0 -- the remaining chunks stream in
behind attention/Q-proj compute instead of serializing in front of it.

Mask modes (host-detected, compile-time): none / causal / mask as before;
"mask" computes the full rectangle (n=512) and multiplies by the 0/1 mask.
"""

import math

import numpy as np
import ml_dtypes

import concourse.bass as bass
import concourse.mybir as mybir
import concourse.tile as tile
from concourse import bacc
from concourse.bass_utils import run_bass_kernel_spmd

F32 = mybir.dt.float32
BF16 = mybir.dt.bfloat16
BF = ml_dtypes.bfloat16

B, S, D = 2, 2048, 2048
H, G = 16, 4
HKV = H // G            # 4
DH = D // H             # 128
DKV = D // G            # 512 (kv projection width)
NCORES = 8
RPC = S // 4            # 512 rows per core
NIC = D // 128          # 16 contraction chunks
NKC = S // 128          # 16 key tiles
SCALE = 1.0 / math.sqrt(DH)
# attention kc order: the K/V all-gather is split in 2 chunks; chunk A
# carries 128-token blocks {0,1} of each rank's quarter (= key tiles
# {4r, 4r+1}), chunk B blocks {2,3}.  Attention runs in two phases: an
# A pass over all head pairs (chunk-A tiles only, partial sums saved to
# SBUF), then a B pass -- so the A pass needs only chunk A, and chunk B
# arrives long before the B pass starts.
CHUNK_BLKS = [(0, 2), (2, 4)]        # [lo, hi) own-token-block range per chunk
KC_A = [4 * b + m for b in range(4) for m in range(2)]
KC_B = [4 * b + m for b in range(4) for m in range(2, 4)]

_NC_CACHE: dict = {}

# set by callers (e.g. test.py) to capture a profile; results of the last run
TRACE = False
TRACE_CORES = None          # e.g. [0] or list(range(8))
LAST_RESULTS = None


def _n_list(mode: str) -> list[int]:
    """Moving-operand width (in q columns, suffix of the 512) per key tile."""
    if mode == "causal":
        return [32 * (16 - kc) for kc in range(NKC)]
    return [512] * NKC


def _build(mode: str):
    mask_mul = mode != "none"
    n_of = _n_list(mode)

    nc = bacc.Bacc("TRN2", target_bir_lowering=False, debug=False,
                   num_devices=NCORES)

    # ---- I/O (host-prepared layouts; all contiguous-DMA friendly) ----
    wq = nc.declare_dram_parameter("wq", [NIC, 128, D], BF16, isOutput=False)
    qt = nc.declare_dram_parameter("qt", [128, NIC * RPC], BF16, isOutput=False)
    # k/v: only this core's 512-token quarter (projected here, all-gathered)
    kt = nc.declare_dram_parameter("kt", [128, NIC * 512], BF16, isOutput=False)
    vt = nc.declare_dram_parameter("vt", [4, 128, NIC * 128], BF16, isOutput=False)
    wk = nc.declare_dram_parameter("wk", [HKV, 128, NIC * 128], BF16, isOutput=False)
    wv = nc.declare_dram_parameter("wv", [128, NIC * DKV], BF16, isOutput=False)
    wo = nc.declare_dram_parameter("wo", [4, 128, H * 512], BF16, isOutput=False)
    cosq = nc.declare_dram_parameter("cosq", [128, RPC], BF16, isOutput=False)
    sinq = nc.declare_dram_parameter("sinq", [128, RPC], BF16, isOutput=False)
    # cos/sin for this core's own k-token quarter
    cosk = nc.declare_dram_parameter("cosk", [128, 512], BF16, isOutput=False)
    sink = nc.declare_dram_parameter("sink", [128, 512], BF16, isOutput=False)
    pswap = nc.declare_dram_parameter("pswap", [128, 128], BF16, isOutput=False)
    if mode == "causal":
        mdiag = nc.declare_dram_parameter("mdiag", [128, 32], BF16,
                                          isOutput=False)
    if mode == "mask":
        m01 = nc.declare_dram_parameter("m01", [128, NKC * RPC], BF16,
                                        isOutput=False)
    out = nc.declare_dram_parameter("out", [RPC, D], F32, isOutput=True)

    with tile.TileContext(nc) as tc:
        with (
            tc.tile_pool(name="res", bufs=1) as res,          # resident
            tc.tile_pool(name="stream2m", bufs=2) as stream2m,  # 2MB blocks
            tc.tile_pool(name="stream05", bufs=7) as stream05,  # 0.5MB blocks
            tc.tile_pool(name="small", bufs=3) as small,
            tc.tile_pool(name="probs", bufs=8) as probsp,
            tc.tile_pool(name="bcast", bufs=2) as bcastp,
            tc.tile_pool(name="dram", bufs=1, space="DRAM") as dramp,
            tc.tile_pool(name="psmm", bufs=5, space="PSUM") as psmm,
            tc.tile_pool(name="psacc", bufs=2, space="PSUM") as psacc,
            tc.tile_pool(name="pssum", bufs=1, space="PSUM") as pssum,
        ):
            # ---------------- resident tiles (DMAs staged per phase) -------
            # K-proj operands stream first so the first matmul fires ASAP;
            # the small constant tiles follow them in the queue.
            kmov = stream2m.tile([128, NIC, 512], BF16, tag="s2m")
            nc.sync.dma_start(
                out=kmov[:, 0:4, :],
                in_=kt[:, 0:4 * 512].rearrange("p (i m) -> p i m", i=4))
            wk0 = stream05.tile([128, NIC, 128], BF16, tag="s05")
            # split so the first matmul only waits on the first quarter
            nc.sync.dma_start(out=wk0[:, 0:4, :], in_=wk[0, :, 0:4 * 128]
                              .rearrange("p (i m) -> p i m", i=4))
            nc.sync.dma_start(out=wk0[:, 4:, :], in_=wk[0, :, 4 * 128:]
                              .rearrange("p (i m) -> p i m", i=NIC - 4))
            for icq in range(1, 4):
                nc.sync.dma_start(
                    out=kmov[:, 4 * icq:4 * icq + 4, :],
                    in_=kt[:, 4 * icq * 512:(4 * icq + 4) * 512].rearrange(
                        "p (i m) -> p i m", i=4))
            coskq_t = res.tile([128, 512], BF16)
            nc.sync.dma_start(out=coskq_t, in_=cosk[:, :])
            sinkq_t = res.tile([128, 512], BF16)
            nc.sync.dma_start(out=sinkq_t, in_=sink[:, :])
            pswap_t = res.tile([128, 128], BF16)
            nc.sync.dma_start(out=pswap_t, in_=pswap[:, :])
            ones_t = res.tile([128, 1], BF16)
            nc.vector.memset(ones_t, 1.0)
            if mode == "causal":
                mdiag_t = res.tile([128, 32], BF16)
                nc.sync.dma_start(out=mdiag_t, in_=mdiag[:, :])
            # Q-proj inputs enqueued early so their DMA descriptors sit
            # ahead of anything that waits on the collectives; only the
            # first quarter of qts goes before the K weights so wk[1..3]
            # aren't stuck behind 2MB of q tokens
            qts = res.tile([128, NIC, RPC], BF16)
            nc.sync.dma_start(out=qts[:, 0:4, :], in_=qt[:, 0:4 * RPC]
                              .rearrange("p (i m) -> p i m", i=4))
            cosq_t = res.tile([128, RPC], BF16)
            nc.sync.dma_start(out=cosq_t, in_=cosq[:, :])
            sinq_t = res.tile([128, RPC], BF16)
            nc.sync.dma_start(out=sinq_t, in_=sinq[:, :])

            qhs = res.tile([128, H, RPC], BF16)     # rope'd q, [dh, h, rows]
            # gathered K/V live in per-chunk tiles so chunk-A consumers
            # only depend on chunk-A unstage DMAs (hazard tracking on one
            # shared tile serializes attention behind the whole gather)
            khsA = res.tile([128, HKV, 4, 256], BF16)
            khsB = res.tile([128, HKV, 4, 256], BF16)
            vhsA = res.tile([128, 8, DKV], BF16)
            vhsB = res.tile([128, 8, DKV], BF16)

            def kv_aps(kc, hk):
                r, m = divmod(kc, 4)
                kt_, vt_ = (khsA, vhsA) if m < 2 else (khsB, vhsB)
                m = m % 2
                return (kt_[:, hk, r, m * 128:(m + 1) * 128],
                        vt_[:, 2 * r + m, hk * 128:(hk + 1) * 128])
            # outu_a shares qts's slot: qts is dead once phase A finishes.
            # split 12/4 so phase D's early matmuls (h<12) don't dep-chain
            # behind the last normalization batch (h>=12).
            outu_a = res.tile([128, 12, RPC], BF16, tag="qts")
            outu_b = res.tile([128, 4, RPC], BF16)

            def outu(h):
                return outu_a[:, h, :] if h < 12 else outu_b[:, h - 12, :]
            # normalization batches: the last batch (latency-exposed right
            # before the out-projection) covers only heads 14-15
            NB = [(0, 8), (8, 14), (14, 16)]
            sums_g = [res.tile([8, RPC], F32, name=f"sums{g}", tag=f"sums{g}")
                      for g in range(len(NB))]
            rec_g = [res.tile([8, RPC], F32, name=f"rec{g}", tag=f"rec{g}")
                     for g in range(len(NB))]
            sums_dram = dramp.tile([16, RPC], F32)
            rec_dram = dramp.tile([16, RPC], F32)
            khs_own = res.tile([128, HKV, 512], BF16)
            vhs_own = res.tile([128, 4, DKV], BF16)
            # 2-chunk staging (2 blocks each); K first, then V
            kv_cinA = dramp.tile([128, 2048], BF16)
            kv_coutA = dramp.tile([4, 128, 2048], BF16)
            kv_cinB = dramp.tile([128, 2048], BF16)
            kv_coutB = dramp.tile([4, 128, 2048], BF16)

            def mm_dedup(out, lhsT, rhs, **kw):
                """Matmul that reuses the PE's already-loaded stationary
                operand (the immediately preceding matmul used the same
                lhsT), skipping the redundant LDWEIGHTS."""
                inst = nc.tensor.matmul(out, lhsT, rhs, **kw)
                try:
                    inst.ins.ldweights = False
                except Exception:
                    pass
                return inst

            def rope(dst, x_bf, ps_pool, cos_ap, sin_ap, n):
                """dst = x*cos + pairswap(x)*sin  (signs baked into sin)."""
                y_ps = ps_pool.tile([128, 512], F32, tag="mm")
                # moving operand max 1024 bf16 per matmul
                assert n <= 512
                nc.tensor.matmul(y_ps[:, :n], pswap_t, x_bf, start=True,
                                 stop=True)
                t1 = small.tile([128, 512], BF16, tag="t1")
                nc.vector.tensor_mul(t1[:, :n], x_bf, cos_ap)
                t2 = small.tile([128, 512], BF16, tag="t2")
                nc.vector.tensor_mul(t2[:, :n], y_ps[:, :n], sin_ap)
                nc.vector.tensor_add(dst, t1[:, :n], t2[:, :n])

            # ------- Phase B: K/V proj for OWN 512-token quarter + RoPE -----
            # (first, so the chunked all-gather overlaps Q proj + attention)
            for hk in range(HKV):
                if hk == 0:
                    wk_all = wk0
                else:
                    wk_all = stream05.tile([128, NIC, 128], BF16, tag="s05")
                    nc.sync.dma_start(out=wk_all, in_=wk[hk].rearrange(
                        "p (i m) -> p i m", i=NIC))
                ps = psmm.tile([128, 512], F32, tag="mm")
                for ic in range(NIC):
                    nc.tensor.matmul(ps, wk_all[:, ic, :],
                                     kmov[:, ic, :],
                                     start=(ic == 0), stop=(ic == NIC - 1))
                xk = small.tile([128, 512], BF16, tag="xq")
                nc.scalar.copy(xk, ps)
                rope(khs_own[:, hk, :], xk, psmm, coskq_t, sinkq_t, 512)

            # rest of qts after the K weights (needed only at Q-proj ~45us)
            nc.sync.dma_start(out=qts[:, 4:, :], in_=qt[:, 4 * RPC:]
                              .rearrange("p (i m) -> p i m", i=NIC - 4))
            wvs = res.tile([128, NIC, DKV], BF16)
            nc.sync.dma_start(out=wvs, in_=wv[:, :].rearrange(
                "p (i n) -> p i n", i=NIC))
            for j in range(4):            # own 128-token blocks (V stationary)
                vmov = stream05.tile([128, NIC, 128], BF16, tag="s05")
                nc.sync.dma_start(out=vmov, in_=vt[j].rearrange(
                    "p (i m) -> p i m", i=NIC))
                ps = psmm.tile([128, 512], F32, tag="mm")
                for ic in range(NIC):
                    nc.tensor.matmul(ps, vmov[:, ic, :],
                                     wvs[:, ic, :],
                                     start=(ic == 0), stop=(ic == NIC - 1))
                nc.vector.tensor_copy(vhs_own[:, j, :], ps)
                if j % 2 == 0:
                    continue
                # stage + all-gather chunk: j==1 -> blocks {0,1} (A),
                # j==3 -> blocks {2,3} (B)
                blo, bhi = CHUNK_BLKS[j // 2]
                nb = bhi - blo
                kv_cin = kv_cinA if j == 1 else kv_cinB
                kv_cout = kv_coutA if j == 1 else kv_coutB
                ksz = HKV * 128 * nb
                nc.sync.dma_start(
                    out=kv_cin[:, 0:ksz].rearrange("p (h m) -> p h m", h=HKV),
                    in_=khs_own[:, :, 128 * blo:128 * bhi])
                nc.sync.dma_start(
                    out=kv_cin[:, ksz:].rearrange("p (v m) -> p v m", v=nb),
                    in_=vhs_own[:, blo:bhi, :])
                nc.gpsimd.collective_compute(
                    "AllGather", mybir.AluOpType.bypass,
                    replica_groups=[[0, 1, 2, 3], [4, 5, 6, 7]],
                    ins=[kv_cin[:, :]], outs=[kv_cout[:, :, :]])
                # NOTE: the unstage DMAs are enqueued AFTER the Q-proj
                # input loads (below) -- their descriptors wait on the CC
                # semaphore and would head-of-line-block every input
                # stream descriptor behind them in the DMA rings.

            def unstage(j):
                blo, bhi = CHUNK_BLKS[j // 2]
                nb = bhi - blo
                kv_cout = kv_coutA if j == 1 else kv_coutB
                ksz = HKV * 128 * nb
                kdst = khsA if j == 1 else khsB
                vdst = vhsA if j == 1 else vhsB
                for r in range(4):
                    nc.sync.dma_start(
                        out=kdst[:, :, r, :],
                        in_=kv_cout[r, :, 0:ksz].rearrange(
                            "p (h x) -> p h x", h=HKV))
                    nc.sync.dma_start(
                        out=vdst[:, 2 * r:2 * r + 2, :],
                        in_=kv_cout[r, :, ksz:].rearrange(
                            "p (v x) -> p v x", v=nb))

            # ---------------- Phase A: Q-proj + RoPE ----------------
            def qproj(oc):
                wq_all = stream05.tile([128, NIC, 128], BF16, tag="s05",
                                       name="wq_all")
                nc.sync.dma_start(out=wq_all, in_=wq[oc].rearrange(
                    "p (i m) -> p i m", i=NIC))
                ps = psmm.tile([128, 512], F32, tag="mm", name="ps_q")
                for ic in range(NIC):
                    nc.tensor.matmul(ps, wq_all[:, ic, :],
                                     qts[:, ic, :],
                                     start=(ic == 0), stop=(ic == NIC - 1))
                xq = small.tile([128, 512], BF16, tag="xq", name="xq")
                nc.scalar.copy(xq, ps)
                rope(qhs[:, oc, :], xq, psmm, cosq_t, sinq_t, RPC)

            for oc in range(12):
                qproj(oc)

            # ---------------- Phase C: attention per head ----------------
            if mode == "mask":
                m01s = res.tile([128, NKC, RPC], BF16)
                nc.sync.dma_start(out=m01s, in_=m01[:, :].rearrange(
                    "p (k m) -> p k m", k=NKC))
            # unstage the gathered K/V chunks (descriptors wait on the CC
            # semaphores -- enqueued after the bulk of the input streams
            # so they don't head-of-line-block the DMA rings)
            unstage(1)

            def normalize_batch(g):
                """reciprocal + broadcast + in-place normalize for the heads
                of batch g (their sums are already in sums_dram)."""
                a, bnd = NB[g]
                m = bnd - a
                nc.sync.dma_start(out=sums_g[g][:m, :],
                                  in_=sums_dram[a:bnd, :])
                nc.vector.reciprocal(rec_g[g][:m, :], sums_g[g][:m, :])
                nc.sync.dma_start(out=rec_dram[a:bnd, :], in_=rec_g[g][:m, :])
                for h in range(a, bnd):
                    recb = bcastp.tile([128, RPC], F32, tag="bc")
                    nc.sync.dma_start(
                        out=recb,
                        in_=rec_dram[h:h + 1, :].to_broadcast([128, RPC]))
                    nc.vector.tensor_mul(outu(h), outu(h), recb)

            LA = 2                # kc-step lookahead (software pipeline)
            # chunk-A partial PV sums (bf16) alias the dead wvs slot;
            # partial row-sums are tiny and get their own tile
            partA_o = res.tile([128, NIC, DKV], BF16, tag="wvs",
                               name="partA_o")
            # per-head partial row-sums: head h at partition 32*(h%4),
            # column slot h//4 (DVE writes must start 32-aligned)
            partA_s = res.tile([128, 4, RPC], F32, name="partA_s")

            def pas(h, c0=0, c1=RPC):
                p = 32 * (h % 4)
                return partA_s[p:p + 1, h // 4, c0:c1]
            # columns below BLO are never written by the B phase (causal:
            # every B-phase kc >= 2 has lo >= 64); they are copied from the
            # A partials instead of added
            BLO = 64 if mode == "causal" else 0

            def attn_pair(hp, kc_list, phase):
                h0, h1 = 2 * hp, 2 * hp + 1
                hk = h0 // G
                nkc = len(kc_list)
                ps_o0 = psacc.tile([128, 512], F32, tag="acc", name="ps_o0")
                ps_o1 = psacc.tile([128, 512], F32, tag="acc", name="ps_o1")
                # both heads' row-sums in one bank: h0 on partition 0,
                # h1 on partition 32 (column-group tiling)
                ps_s = pssum.tile([128, 512], F32, tag="sum", name="ps_s")
                pending = {}

                def issue_scores(i):
                    kc = kc_list[i]
                    n = n_of[kc]
                    lo = RPC - n          # suffix columns
                    kap, _ = kv_aps(kc, hk)
                    packed = 2 * n <= 512
                    if packed:
                        # one matmul computes both heads' scores: the moving
                        # operand is the 3D slice [128, 2 heads, n]
                        ps_sc = psmm.tile([128, 512], F32, tag="mm")
                        nc.tensor.matmul(ps_sc[:, :2 * n], kap,
                                         qhs[:, h0:h0 + 2, lo:],
                                         start=True, stop=True,
                                         skip_group_check=True)
                        probs = probsp.tile([128, 512], BF16, tag="pr")
                        nc.scalar.activation(
                            probs[:, :2 * n], ps_sc[:, :2 * n],
                            mybir.ActivationFunctionType.Exp, scale=SCALE)
                        pr0, pr1 = probs[:, :n], probs[:, n:2 * n]
                        d0, d1 = probs[:, 0:32], probs[:, n:n + 32]
                    else:
                        ps_a = psmm.tile([128, 512], F32, tag="mm")
                        nc.tensor.matmul(ps_a[:, :n], kap, qhs[:, h0, lo:],
                                         start=True, stop=True,
                                         skip_group_check=True)
                        ps_b = psmm.tile([128, 512], F32, tag="mm")
                        mm_dedup(ps_b[:, :n], kap, qhs[:, h1, lo:],
                                 start=True, stop=True,
                                 skip_group_check=True)
                        probs0 = probsp.tile([128, 512], BF16, tag="pr")
                        nc.scalar.activation(
                            probs0[:, :n], ps_a[:, :n],
                            mybir.ActivationFunctionType.Exp, scale=SCALE)
                        probs1 = probsp.tile([128, 512], BF16, tag="pr")
                        nc.scalar.activation(
                            probs1[:, :n], ps_b[:, :n],
                            mybir.ActivationFunctionType.Exp, scale=SCALE)
                        pr0, pr1 = probs0[:, :n], probs1[:, :n]
                        d0, d1 = probs0[:, 0:32], probs1[:, 0:32]
                    if mode == "causal":
                        # only the first 32 suffix columns (the diagonal
                        # 32-row q-block) are partially masked
                        nc.vector.tensor_mul(d0, d0, mdiag_t)
                        nc.vector.tensor_mul(d1, d1, mdiag_t)
                    elif mask_mul:
                        map_ = m01s[:, kc, lo:]
                        nc.vector.tensor_mul(pr0, pr0, map_)
                        nc.vector.tensor_mul(pr1, pr1, map_)
                    pending[i] = (pr0, pr1, kc, n, lo)

                for i in range(LA):
                    issue_scores(i)
                for idx in range(nkc):
                    if idx + LA < nkc:
                        issue_scores(idx + LA)
                    pr0, pr1, kc, n, lo = pending.pop(idx)
                    first = idx == 0
                    last = idx == nkc - 1
                    _, vap = kv_aps(kc, hk)
                    # the two row-sum matmuls target different column
                    # groups of the PE array and run concurrently
                    nc.tensor.matmul(ps_s[0:1, lo:], ones_t, pr0,
                                     start=first, stop=last,
                                     skip_group_check=True,
                                     tile_position=(0, 0))
                    nc.tensor.matmul(ps_s[32:33, lo:], ones_t, pr1,
                                     start=first, stop=last,
                                     skip_group_check=True,
                                     tile_position=(0, 32))
                    nc.tensor.matmul(ps_o0[:, lo:], vap, pr0,
                                     start=first, stop=last,
                                     skip_group_check=True)
                    mm_dedup(ps_o1[:, lo:], vap, pr1,
                             start=first, stop=last,
                             skip_group_check=True)
                for h, strip, ps_o in ((h0, 0, ps_o0), (h1, 32, ps_o1)):
                    if phase == "A":
                        # save chunk-A partials; B pass completes them
                        nc.vector.tensor_copy(partA_o[:, h, :], ps_o)
                        nc.vector.tensor_copy(pas(h),
                                              ps_s[strip:strip + 1, :])
                        continue
                    if phase == "F":
                        # single full pass (both chunks were already
                        # gathered when this pair started)
                        sm1 = small.tile([1, RPC], F32, tag="sm1", bufs=2)
                        nc.vector.tensor_copy(sm1, ps_s[strip:strip + 1, :])
                        nc.sync.dma_start(out=sums_dram[h:h + 1, :], in_=sm1)
                        nc.vector.tensor_copy(outu(h), ps_o)
                        continue
                    sm1 = small.tile([1, RPC], F32, tag="sm1", bufs=2)
                    if BLO:
                        nc.vector.tensor_copy(sm1[:, :BLO], pas(h, 0, BLO))
                        nc.vector.tensor_copy(
                            outu_a[:, h, :BLO] if h < 12
                            else outu_b[:, h - 12, :BLO],
                            partA_o[:, h, :BLO])
                    nc.vector.tensor_add(sm1[:, BLO:], pas(h, BLO, RPC),
                                         ps_s[strip:strip + 1, BLO:])
                    nc.sync.dma_start(out=sums_dram[h:h + 1, :], in_=sm1)
                    nc.vector.tensor_add(
                        outu_a[:, h, BLO:] if h < 12
                        else outu_b[:, h - 12, BLO:],
                        partA_o[:, h, BLO:], ps_o[:, BLO:])

            # the first two A pairs interleave with the tail of Q-proj so
            # ScalarE's exp stream starts as soon as chunk A lands
            attn_pair(0, KC_A, "A")
            attn_pair(1, KC_A, "A")
            for oc in range(12, 16):
                qproj(oc)
            unstage(3)
            for hp in range(2, H // 2):
                attn_pair(hp, KC_A, "A")
            for hp in range(H // 2):
                attn_pair(hp, KC_B, "B")
                if hp == 3:
                    normalize_batch(0)
                elif hp == 6:
                    normalize_batch(1)
            normalize_batch(2)

            # ---------------- Phase D: out-projection ----------------
            for oc in range(4):
                wo_all = stream2m.tile([128, H, 512], BF16, tag="s2m")
                nc.sync.dma_start(out=wo_all, in_=wo[oc].rearrange(
                    "p (h m) -> p h m", h=H))
                for qc in range(4):
                    ps_f = psmm.tile([128, 512], F32, tag="mm")
                    for h in range(H):
                        lh = outu_a[:, h, qc * 128:(qc + 1) * 128] if h < 12 \
                            else outu_b[:, h - 12, qc * 128:(qc + 1) * 128]
                        nc.tensor.matmul(
                            ps_f, lh, wo_all[:, h, :],
                            start=(h == 0), stop=(h == H - 1))
                    fin = small.tile([128, 512], F32, tag="fin")
                    nc.vector.tensor_copy(fin, ps_f)
                    nc.sync.dma_start(
                        out=out[qc * 128:(qc + 1) * 128,
                                oc * 512:(oc + 1) * 512],
                        in_=fin)

    nc.compile()
    return nc


def _get_nc(mode: str):
    if mode not in _NC_CACHE:
        _NC_CACHE[mode] = _build(mode)
    return _NC_CACHE[mode]


def _core_rows(mode: str, r: int) -> np.ndarray:
    """Global (within-batch) q-row indices owned by quarter r, ascending.

    causal: 16 interleaved 32-row blocks {4j + r : j} -> exact suffix
    causality, identical shapes on every core.  other modes: contiguous.
    """
    if mode == "causal":
        return np.concatenate([np.arange(32 * (4 * j + r), 32 * (4 * j + r + 1))
                               for j in range(16)])
    return np.arange(r * RPC, (r + 1) * RPC)


def kernel(q, k, v, mask, freqs, W_q, W_k, W_v, W_o):
    q = np.asarray(q, dtype=np.float32)
    k = np.asarray(k, dtype=np.float32)
    v = np.asarray(v, dtype=np.float32)
    mask = np.asarray(mask, dtype=np.float32)
    freqs = np.asarray(freqs, dtype=np.float32)
    W_q = np.asarray(W_q, dtype=np.float32)
    W_k = np.asarray(W_k, dtype=np.float32)
    W_v = np.asarray(W_v, dtype=np.float32)
    W_o = np.asarray(W_o, dtype=np.float32)

    # ---- mask mode detection ----
    nz = mask != 0
    if nz.all():
        mode = "none"
    else:
        tril = np.tril(np.ones((S, S), dtype=bool))
        mode = "causal" if all(np.array_equal(nz[b], tril) for b in range(B)) \
            else "mask"

    # ---- shared host precomputation ----
    c_full = np.cos(freqs)                      # [S, 64]
    s_full = np.sin(freqs)
    sgn = np.tile(np.array([-1.0, 1.0], np.float32), DH // 2)  # [-,+,-,+...]
    cosk_h = np.repeat(c_full, 2, axis=1).T.astype(BF)          # [128, S]
    sink_h = (np.repeat(s_full, 2, axis=1) * sgn).T.astype(BF)

    psw = np.zeros((128, 128), np.float32)
    idx = np.arange(128)
    psw[idx, idx ^ 1] = 1.0
    psw = psw.astype(BF)

    # weight layouts
    # wq[oc, p, i*128+m] = W_q[oc*128+m, i*128+p]
    wq_h = np.ascontiguousarray(
        W_q.reshape(H, 128, NIC, 128).transpose(0, 3, 2, 1)
        .reshape(H, 128, D)).astype(BF)
    # wk[hk, p, i*128+m] = W_k[hk*128+m, i*128+p]
    wk_h = np.ascontiguousarray(
        W_k.reshape(HKV, 128, NIC, 128).transpose(0, 3, 2, 1)
        .reshape(HKV, 128, D)).astype(BF)
    # wv[p, i*512+n] = W_v[n, i*128+p]
    wv_h = np.ascontiguousarray(
        W_v.reshape(DKV, NIC, 128).transpose(2, 1, 0).reshape(128, NIC * DKV)
    ).astype(BF)
    # wo[oc, p, h*512+m] = W_o[oc*512+m, h*128+p]
    wo_h = np.ascontiguousarray(
        W_o.reshape(4, 512, H, 128).transpose(0, 3, 2, 1).reshape(4, 128, -1)
    ).astype(BF)

    # k/v: each core only gets its own 512-token quarter (gathered on device)
    # kt[p, i*512+t] = k[b, tq*512+t, i*128+p] for quarter tq
    kt_b = []   # [B][4] quarters
    vt_b = []
    for b in range(B):
        kt_b.append([np.ascontiguousarray(
            k[b, tq * 512:(tq + 1) * 512].reshape(512, NIC, 128)
            .transpose(2, 1, 0).reshape(128, NIC * 512)).astype(BF)
            for tq in range(4)])
        # vt[j, p, i*128+t] = v[b, tq*512 + j*128+t, i*128+p]
        vt_b.append([np.ascontiguousarray(
            v[b, tq * 512:(tq + 1) * 512].reshape(4, 128, NIC, 128)
            .transpose(0, 3, 2, 1).reshape(4, 128, NIC * 128)).astype(BF)
            for tq in range(4)])

    in_maps = []
    rows_all = []
    for c in range(NCORES):
        b, r = divmod(c, 4)
        rows = _core_rows(mode, r)
        rows_all.append((b, rows))
        # qt[p, i*512+t] = q[b, rows[t], i*128+p]
        qsl = q[b][rows]                       # [512, D]
        qt_h = np.ascontiguousarray(
            qsl.reshape(RPC, NIC, 128).transpose(2, 1, 0).reshape(128, -1)
        ).astype(BF)
        cq = np.repeat(c_full[rows], 2, axis=1).T.astype(BF)      # [128, 512]
        sq = (np.repeat(s_full[rows], 2, axis=1) * sgn).T.astype(BF)
        im = {
            "wq": wq_h, "qt": qt_h, "kt": kt_b[b][r], "vt": vt_b[b][r],
            "wk": wk_h, "wv": wv_h, "wo": wo_h,
            "cosq": cq, "sinq": sq,
            "cosk": np.ascontiguousarray(cosk_h[:, r * 512:(r + 1) * 512]),
            "sink": np.ascontiguousarray(sink_h[:, r * 512:(r + 1) * 512]),
            "pswap": psw,
        }
        if mode == "causal":
            # diagonal 32-col block mask: keep key p of tile kc for local
            # row i of block j=kc  <=>  p <= 32*r + i  (same for all kc)
            pp = np.arange(128)[:, None]
            ii = np.arange(32)[None, :]
            im["mdiag"] = (pp <= 32 * r + ii).astype(BF)
        elif mode == "mask":
            # m01[p, kc*512+m] = (mask[b, rows[m], kc*128+p] != 0)
            msl = nz[b][rows]                  # [512, S] bool
            m01_h = np.ascontiguousarray(
                msl.T.reshape(NKC, 128, RPC).transpose(1, 0, 2)
                .reshape(128, -1)).astype(BF)
            im["m01"] = m01_h
        in_maps.append(im)

    nc = _get_nc(mode)
    kwargs = {}
    if TRACE:
        kwargs["trace"] = True
        if TRACE_CORES:
            kwargs["trace_cores"] = list(TRACE_CORES)
    results = run_bass_kernel_spmd(nc, in_maps, core_ids=list(range(NCORES)),
                                   **kwargs)
    global LAST_RESULTS
    LAST_RESULTS = results

    full = np.empty((B, S, D), np.float32)
    for c in range(NCORES):
        b, rows = rows_all[c]
        full[b, rows] = results.results[c]["out"]
    return full


# revision 73
# speedup vs baseline: 1.0414x; 1.0117x over previous
"""Trainium2 Bass kernel for MultiHead GQA attention (B=2, S=2048, D=2048,
H=16 query heads, HKV=4 kv heads, DH=128, RoPE, mask, out-proj).

Sharding: token-parallel across 8 cores. Core c handles batch c//4 and 512
query rows of it. Each core projects K/V for its own 512-token quarter
(all 4 kv heads), the quarters are all-gathered in 4 pipelined 128-token
chunks, and the core runs attention + out-proj for its rows. Host
reassembles. All matmuls bf16 with fp32 PSUM accumulation.

Causal handling (exact, SPMD-uniform): core r of its batch owns the 16
interleaved 32-row q-blocks {4j + r : j=0..15} (ascending). For key tile
kc the q-blocks that attend to it are exactly the suffix of blocks with
position j >= kc, i.e. a contiguous column suffix of width n = 32*(16-kc)
-- identical on every core. Only the first 32 columns of each suffix (the
diagonal block) are partially masked; they get multiplied by a per-core
[128, 32] 0/1 tile. This computes 34 128x128-tile-equivalents per head
(the exact causal minimum for a 4-way row split) vs 40 for the previous
128-row-block scheme.

Attention is computed transposed: scoresT[keys, q] = khT.T @ qhT per
128-key tile, exp on ScalarE (scale folded in), probs bf16, then
outT[dh, q] += v_tile.T @ probsT, and row-sums via a ones-stationary
matmul. outT feeds the out-projection directly as stationary operand.

The K/V all-gather is split into 4 collectives, one per 128-token block
of each rank's quarter: chunk m delivers key tiles {4r + m : r=0..3}.
Attention iterates kc in the order [0,4,8,12, 1,5,9,13, ...] so the
first kc group only needs chunk 0 -- the remaining chunks stream in
behind attention/Q-proj compute instead of serializing in front of it.

Mask modes (host-detected, compile-time): none / causal / mask as before;
"mask" computes the full rectangle (n=512) and multiplies by the 0/1 mask.
"""

import math

import numpy as np
import ml_dtypes

import concourse.bass as bass
import concourse.mybir as mybir
import concourse.tile as tile
from concourse import bacc
from concourse.bass_utils import run_bass_kernel_spmd

F32 = mybir.dt.float32
BF16 = mybir.dt.bfloat16
BF = ml_dtypes.bfloat16

B, S, D = 2, 2048, 2048
H, G = 16, 4
HKV = H // G            # 4
DH = D // H             # 128
DKV = D // G            # 512 (kv projection width)
NCORES = 8
RPC = S // 4            # 512 rows per core
NIC = D // 128          # 16 contraction chunks
NKC = S // 128          # 16 key tiles
SCALE = 1.0 / math.sqrt(DH)
# attention kc order: the K/V all-gather is split in 2 chunks; chunk A
# carries 128-token blocks {0,1} of each rank's quarter (= key tiles
# {4r, 4r+1}), chunk B blocks {2,3}.  Attention runs in two phases: an
# A pass over all head pairs (chunk-A tiles only, partial sums saved to
# SBUF), then a B pass -- so the A pass needs only chunk A, and chunk B
# arrives long before the B pass starts.
CHUNK_BLKS = [(0, 2), (2, 4)]        # [lo, hi) own-token-block range per chunk
KC_A = [4 * b + m for b in range(4) for m in range(2)]
KC_B = [4 * b + m for b in range(4) for m in range(2, 4)]

_NC_CACHE: dict = {}

# set by callers (e.g. test.py) to capture a profile; results of the last run
TRACE = False
TRACE_CORES = None          # e.g. [0] or list(range(8))
LAST_RESULTS = None


def _n_list(mode: str) -> list[int]:
    """Moving-operand width (in q columns, suffix of the 512) per key tile."""
    if mode == "causal":
        return [32 * (16 - kc) for kc in range(NKC)]
    return [512] * NKC


def _build(mode: str):
    mask_mul = mode != "none"
    n_of = _n_list(mode)

    nc = bacc.Bacc("TRN2", target_bir_lowering=False, debug=False,
                   num_devices=NCORES)

    # ---- I/O (host-prepared layouts; all contiguous-DMA friendly) ----
    wq = nc.declare_dram_parameter("wq", [NIC, 128, D], BF16, isOutput=False)
    qt = nc.declare_dram_parameter("qt", [128, NIC * RPC], BF16, isOutput=False)
    # k/v: only this core's 512-token quarter (projected here, all-gathered)
    kt = nc.declare_dram_parameter("kt", [128, NIC * 512], BF16, isOutput=False)
    vt = nc.declare_dram_parameter("vt", [4, 128, NIC * 128], BF16, isOutput=False)
    wk = nc.declare_dram_parameter("wk", [HKV, 128, NIC * 128], BF16, isOutput=False)
    wv = nc.declare_dram_parameter("wv", [128, NIC * DKV], BF16, isOutput=False)
    wo = nc.declare_dram_parameter("wo", [4, 128, H * 512], BF16, isOutput=False)
    cosq = nc.declare_dram_parameter("cosq", [128, RPC], BF16, isOutput=False)
    sinq = nc.declare_dram_parameter("sinq", [128, RPC], BF16, isOutput=False)
    # cos/sin for this core's own k-token quarter
    cosk = nc.declare_dram_parameter("cosk", [128, 512], BF16, isOutput=False)
    sink = nc.declare_dram_parameter("sink", [128, 512], BF16, isOutput=False)
    pswap = nc.declare_dram_parameter("pswap", [128, 128], BF16, isOutput=False)
    if mode == "causal":
        mdiag = nc.declare_dram_parameter("mdiag", [128, 32], BF16,
                                          isOutput=False)
    if mode == "mask":
        m01 = nc.declare_dram_parameter("m01", [128, NKC * RPC], BF16,
                                        isOutput=False)
    out = nc.declare_dram_parameter("out", [RPC, D], F32, isOutput=True)

    with tile.TileContext(nc) as tc:
        with (
            tc.tile_pool(name="res", bufs=1) as res,          # resident
            tc.tile_pool(name="stream2m", bufs=2) as stream2m,  # 2MB blocks
            tc.tile_pool(name="stream05", bufs=7) as stream05,  # 0.5MB blocks
            tc.tile_pool(name="small", bufs=3) as small,
            tc.tile_pool(name="probs", bufs=8) as probsp,
            tc.tile_pool(name="bcast", bufs=2) as bcastp,
            tc.tile_pool(name="dram", bufs=1, space="DRAM") as dramp,
            tc.tile_pool(name="psmm", bufs=5, space="PSUM") as psmm,
            tc.tile_pool(name="psacc", bufs=2, space="PSUM") as psacc,
            tc.tile_pool(name="pssum", bufs=1, space="PSUM") as pssum,
        ):
            # ---------------- resident tiles (DMAs staged per phase) -------
            # K-proj operands stream first so the first matmul fires ASAP;
            # the small constant tiles follow them in the queue.
            kmov = stream2m.tile([128, NIC, 512], BF16, tag="s2m")
            nc.sync.dma_start(
                out=kmov[:, 0:4, :],
                in_=kt[:, 0:4 * 512].rearrange("p (i m) -> p i m", i=4))
            wk0 = stream05.tile([128, NIC, 128], BF16, tag="s05")
            # split so the first matmul only waits on the first quarter
            nc.sync.dma_start(out=wk0[:, 0:4, :], in_=wk[0, :, 0:4 * 128]
                              .rearrange("p (i m) -> p i m", i=4))
            nc.sync.dma_start(out=wk0[:, 4:, :], in_=wk[0, :, 4 * 128:]
                              .rearrange("p (i m) -> p i m", i=NIC - 4))
            for icq in range(1, 4):
                nc.sync.dma_start(
                    out=kmov[:, 4 * icq:4 * icq + 4, :],
                    in_=kt[:, 4 * icq * 512:(4 * icq + 4) * 512].rearrange(
                        "p (i m) -> p i m", i=4))
            coskq_t = res.tile([128, 512], BF16)
            nc.sync.dma_start(out=coskq_t, in_=cosk[:, :])
            sinkq_t = res.tile([128, 512], BF16)
            nc.sync.dma_start(out=sinkq_t, in_=sink[:, :])
            pswap_t = res.tile([128, 128], BF16)
            nc.sync.dma_start(out=pswap_t, in_=pswap[:, :])
            ones_t = res.tile([128, 1], BF16)
            nc.vector.memset(ones_t, 1.0)
            if mode == "causal":
                mdiag_t = res.tile([128, 32], BF16)
                nc.sync.dma_start(out=mdiag_t, in_=mdiag[:, :])
            # remaining K-proj weights hoisted ahead of the Q-side loads
            # (needed at ~15-21us, long before qts/cosq/sinq at ~45us)
            wk_t = [wk0]
            for hk in range(1, HKV):
                wkt = stream05.tile([128, NIC, 128], BF16, tag="s05",
                                    name=f"wk{hk}")
                nc.sync.dma_start(out=wkt, in_=wk[hk].rearrange(
                    "p (i m) -> p i m", i=NIC))
                wk_t.append(wkt)
            # Q-proj inputs enqueued early so their DMA descriptors sit
            # ahead of anything that waits on the collectives; only the
            # first quarter of qts goes before the K weights so wk[1..3]
            # aren't stuck behind 2MB of q tokens
            qts = res.tile([128, NIC, RPC], BF16)
            nc.sync.dma_start(out=qts[:, 0:4, :], in_=qt[:, 0:4 * RPC]
                              .rearrange("p (i m) -> p i m", i=4))
            cosq_t = res.tile([128, RPC], BF16)
            nc.sync.dma_start(out=cosq_t, in_=cosq[:, :])
            sinq_t = res.tile([128, RPC], BF16)
            nc.sync.dma_start(out=sinq_t, in_=sinq[:, :])

            qhs = res.tile([128, H, RPC], BF16)     # rope'd q, [dh, h, rows]
            # gathered K/V live in per-chunk tiles so chunk-A consumers
            # only depend on chunk-A unstage DMAs (hazard tracking on one
            # shared tile serializes attention behind the whole gather)
            khsA = res.tile([128, HKV, 4, 256], BF16)
            khsB = res.tile([128, HKV, 4, 256], BF16)
            vhsA = res.tile([128, 8, DKV], BF16)
            vhsB = res.tile([128, 8, DKV], BF16)

            def kv_aps(kc, hk):
                r, m = divmod(kc, 4)
                kt_, vt_ = (khsA, vhsA) if m < 2 else (khsB, vhsB)
                m = m % 2
                return (kt_[:, hk, r, m * 128:(m + 1) * 128],
                        vt_[:, 2 * r + m, hk * 128:(hk + 1) * 128])
            # outu_a shares qts's slot: qts is dead once phase A finishes.
            # split 12/4 so phase D's early matmuls (h<12) don't dep-chain
            # behind the last normalization batch (h>=12).
            outu_a = res.tile([128, 12, RPC], BF16, tag="qts")
            outu_b = res.tile([128, 4, RPC], BF16)

            def outu(h):
                return outu_a[:, h, :] if h < 12 else outu_b[:, h - 12, :]
            # normalization batches: the last batch (latency-exposed right
            # before the out-projection) covers only heads 14-15
            NB = [(0, 8), (8, 14), (14, 16)]
            sums_g = [res.tile([8, RPC], F32, name=f"sums{g}", tag=f"sums{g}")
                      for g in range(len(NB))]
            rec_g = [res.tile([8, RPC], F32, name=f"rec{g}", tag=f"rec{g}")
                     for g in range(len(NB))]
            sums_dram = dramp.tile([16, RPC], F32)
            rec_dram = dramp.tile([16, RPC], F32)
            khs_own = res.tile([128, HKV, 512], BF16)
            vhs_own = res.tile([128, 4, DKV], BF16)
            # 2-chunk staging (2 blocks each); K first, then V
            kv_cinA = dramp.tile([128, 2048], BF16)
            kv_coutA = dramp.tile([4, 128, 2048], BF16)
            kv_cinB = dramp.tile([128, 2048], BF16)
            kv_coutB = dramp.tile([4, 128, 2048], BF16)

            def mm_dedup(out, lhsT, rhs, **kw):
                """Matmul that reuses the PE's already-loaded stationary
                operand (the immediately preceding matmul used the same
                lhsT), skipping the redundant LDWEIGHTS."""
                inst = nc.tensor.matmul(out, lhsT, rhs, **kw)
                try:
                    inst.ins.ldweights = False
                except Exception:
                    pass
                return inst

            def rope(dst, x_bf, ps_pool, cos_ap, sin_ap, n):
                """dst = x*cos + pairswap(x)*sin  (signs baked into sin)."""
                y_ps = ps_pool.tile([128, 512], F32, tag="mm")
                # moving operand max 1024 bf16 per matmul
                assert n <= 512
                nc.tensor.matmul(y_ps[:, :n], pswap_t, x_bf, start=True,
                                 stop=True)
                t1 = small.tile([128, 512], BF16, tag="t1")
                nc.vector.tensor_mul(t1[:, :n], x_bf, cos_ap)
                t2 = small.tile([128, 512], BF16, tag="t2")
                nc.vector.tensor_mul(t2[:, :n], y_ps[:, :n], sin_ap)
                nc.vector.tensor_add(dst, t1[:, :n], t2[:, :n])

            # ------- Phase B: K/V proj for OWN 512-token quarter + RoPE -----
            # (first, so the chunked all-gather overlaps Q proj + attention)
            for hk in range(HKV):
                wk_all = wk_t[hk]
                ps = psmm.tile([128, 512], F32, tag="mm")
                for ic in range(NIC):
                    nc.tensor.matmul(ps, wk_all[:, ic, :],
                                     kmov[:, ic, :],
                                     start=(ic == 0), stop=(ic == NIC - 1))
                xk = small.tile([128, 512], BF16, tag="xq")
                nc.scalar.copy(xk, ps)
                rope(khs_own[:, hk, :], xk, psmm, coskq_t, sinkq_t, 512)

            # rest of qts after the K weights (needed only at Q-proj ~45us)
            nc.sync.dma_start(out=qts[:, 4:, :], in_=qt[:, 4 * RPC:]
                              .rearrange("p (i m) -> p i m", i=NIC - 4))
            wvs = res.tile([128, NIC, DKV], BF16)
            nc.sync.dma_start(out=wvs, in_=wv[:, :].rearrange(
                "p (i n) -> p i n", i=NIC))
            for j in range(4):            # own 128-token blocks (V stationary)
                vmov = stream05.tile([128, NIC, 128], BF16, tag="s05")
                nc.sync.dma_start(out=vmov, in_=vt[j].rearrange(
                    "p (i m) -> p i m", i=NIC))
                ps = psmm.tile([128, 512], F32, tag="mm")
                for ic in range(NIC):
                    nc.tensor.matmul(ps, vmov[:, ic, :],
                                     wvs[:, ic, :],
                                     start=(ic == 0), stop=(ic == NIC - 1))
                nc.vector.tensor_copy(vhs_own[:, j, :], ps)
                if j % 2 == 0:
                    continue
                # stage + all-gather chunk: j==1 -> blocks {0,1} (A),
                # j==3 -> blocks {2,3} (B)
                blo, bhi = CHUNK_BLKS[j // 2]
                nb = bhi - blo
                kv_cin = kv_cinA if j == 1 else kv_cinB
                kv_cout = kv_coutA if j == 1 else kv_coutB
                ksz = HKV * 128 * nb
                nc.sync.dma_start(
                    out=kv_cin[:, 0:ksz].rearrange("p (h m) -> p h m", h=HKV),
                    in_=khs_own[:, :, 128 * blo:128 * bhi])
                nc.sync.dma_start(
                    out=kv_cin[:, ksz:].rearrange("p (v m) -> p v m", v=nb),
                    in_=vhs_own[:, blo:bhi, :])
                nc.gpsimd.collective_compute(
                    "AllGather", mybir.AluOpType.bypass,
                    replica_groups=[[0, 1, 2, 3], [4, 5, 6, 7]],
                    ins=[kv_cin[:, :]], outs=[kv_cout[:, :, :]])
                # NOTE: the unstage DMAs are enqueued AFTER the Q-proj
                # input loads (below) -- their descriptors wait on the CC
                # semaphore and would head-of-line-block every input
                # stream descriptor behind them in the DMA rings.

            def unstage(j):
                blo, bhi = CHUNK_BLKS[j // 2]
                nb = bhi - blo
                kv_cout = kv_coutA if j == 1 else kv_coutB
                ksz = HKV * 128 * nb
                kdst = khsA if j == 1 else khsB
                vdst = vhsA if j == 1 else vhsB
                for r in range(4):
                    nc.sync.dma_start(
                        out=kdst[:, :, r, :],
                        in_=kv_cout[r, :, 0:ksz].rearrange(
                            "p (h x) -> p h x", h=HKV))
                    nc.sync.dma_start(
                        out=vdst[:, 2 * r:2 * r + 2, :],
                        in_=kv_cout[r, :, ksz:].rearrange(
                            "p (v x) -> p v x", v=nb))

            # ---------------- Phase A: Q-proj + RoPE ----------------
            def qproj(oc):
                wq_all = stream05.tile([128, NIC, 128], BF16, tag="s05",
                                       name="wq_all")
                nc.sync.dma_start(out=wq_all, in_=wq[oc].rearrange(
                    "p (i m) -> p i m", i=NIC))
                ps = psmm.tile([128, 512], F32, tag="mm", name="ps_q")
                for ic in range(NIC):
                    nc.tensor.matmul(ps, wq_all[:, ic, :],
                                     qts[:, ic, :],
                                     start=(ic == 0), stop=(ic == NIC - 1))
                xq = small.tile([128, 512], BF16, tag="xq", name="xq")
                nc.scalar.copy(xq, ps)
                rope(qhs[:, oc, :], xq, psmm, cosq_t, sinq_t, RPC)

            for oc in range(12):
                qproj(oc)

            # ---------------- Phase C: attention per head ----------------
            if mode == "mask":
                m01s = res.tile([128, NKC, RPC], BF16)
                nc.sync.dma_start(out=m01s, in_=m01[:, :].rearrange(
                    "p (k m) -> p k m", k=NKC))
            # unstage the gathered K/V chunks (descriptors wait on the CC
            # semaphores -- enqueued after the bulk of the input streams
            # so they don't head-of-line-block the DMA rings)
            unstage(1)

            def normalize_batch(g):
                """reciprocal + broadcast + in-place normalize for the heads
                of batch g (their sums are already in sums_dram)."""
                a, bnd = NB[g]
                m = bnd - a
                nc.sync.dma_start(out=sums_g[g][:m, :],
                                  in_=sums_dram[a:bnd, :])
                nc.vector.reciprocal(rec_g[g][:m, :], sums_g[g][:m, :])
                nc.sync.dma_start(out=rec_dram[a:bnd, :], in_=rec_g[g][:m, :])
                for h in range(a, bnd):
                    recb = bcastp.tile([128, RPC], F32, tag="bc")
                    nc.sync.dma_start(
                        out=recb,
                        in_=rec_dram[h:h + 1, :].to_broadcast([128, RPC]))
                    nc.vector.tensor_mul(outu(h), outu(h), recb)

            LA = 2                # kc-step lookahead (software pipeline)
            # chunk-A partial PV sums (bf16) alias the dead wvs slot;
            # partial row-sums are tiny and get their own tile
            partA_o = res.tile([128, NIC, DKV], BF16, tag="wvs",
                               name="partA_o")
            # per-head partial row-sums: head h at partition 32*(h%4),
            # column slot h//4 (DVE writes must start 32-aligned)
            partA_s = res.tile([128, 4, RPC], F32, name="partA_s")

            def pas(h, c0=0, c1=RPC):
                p = 32 * (h % 4)
                return partA_s[p:p + 1, h // 4, c0:c1]
            # columns below BLO are never written by the B phase (causal:
            # every B-phase kc >= 2 has lo >= 64); they are copied from the
            # A partials instead of added
            BLO = 64 if mode == "causal" else 0

            def attn_pair(hp, kc_list, phase):
                h0, h1 = 2 * hp, 2 * hp + 1
                hk = h0 // G
                nkc = len(kc_list)
                ps_o0 = psacc.tile([128, 512], F32, tag="acc", name="ps_o0")
                ps_o1 = psacc.tile([128, 512], F32, tag="acc", name="ps_o1")
                # both heads' row-sums in one bank: h0 on partition 0,
                # h1 on partition 32 (column-group tiling)
                ps_s = pssum.tile([128, 512], F32, tag="sum", name="ps_s")
                pending = {}

                def issue_scores(i):
                    kc = kc_list[i]
                    n = n_of[kc]
                    lo = RPC - n          # suffix columns
                    kap, _ = kv_aps(kc, hk)
                    packed = 2 * n <= 512
                    if packed:
                        # one matmul computes both heads' scores: the moving
                        # operand is the 3D slice [128, 2 heads, n]
                        ps_sc = psmm.tile([128, 512], F32, tag="mm")
                        nc.tensor.matmul(ps_sc[:, :2 * n], kap,
                                         qhs[:, h0:h0 + 2, lo:],
                                         start=True, stop=True,
                                         skip_group_check=True)
                        probs = probsp.tile([128, 512], BF16, tag="pr")
                        nc.scalar.activation(
                            probs[:, :2 * n], ps_sc[:, :2 * n],
                            mybir.ActivationFunctionType.Exp, scale=SCALE)
                        pr0, pr1 = probs[:, :n], probs[:, n:2 * n]
                        d0, d1 = probs[:, 0:32], probs[:, n:n + 32]
                    else:
                        ps_a = psmm.tile([128, 512], F32, tag="mm")
                        nc.tensor.matmul(ps_a[:, :n], kap, qhs[:, h0, lo:],
                                         start=True, stop=True,
                                         skip_group_check=True)
                        ps_b = psmm.tile([128, 512], F32, tag="mm")
                        mm_dedup(ps_b[:, :n], kap, qhs[:, h1, lo:],
                                 start=True, stop=True,
                                 skip_group_check=True)
                        probs0 = probsp.tile([128, 512], BF16, tag="pr")
                        nc.scalar.activation(
                            probs0[:, :n], ps_a[:, :n],
                            mybir.ActivationFunctionType.Exp, scale=SCALE)
                        probs1 = probsp.tile([128, 512], BF16, tag="pr")
                        nc.scalar.activation(
                            probs1[:, :n], ps_b[:, :n],
                            mybir.ActivationFunctionType.Exp, scale=SCALE)
                        pr0, pr1 = probs0[:, :n], probs1[:, :n]
                        d0, d1 = probs0[:, 0:32], probs1[:, 0:32]
                    if mode == "causal":
                        # only the first 32 suffix columns (the diagonal
                        # 32-row q-block) are partially masked
                        nc.vector.tensor_mul(d0, d0, mdiag_t)
                        nc.vector.tensor_mul(d1, d1, mdiag_t)
                    elif mask_mul:
                        map_ = m01s[:, kc, lo:]
                        nc.vector.tensor_mul(pr0, pr0, map_)
                        nc.vector.tensor_mul(pr1, pr1, map_)
                    pending[i] = (pr0, pr1, kc, n, lo)

                for i in range(LA):
                    issue_scores(i)
                for idx in range(nkc):
                    if idx + LA < nkc:
                        issue_scores(idx + LA)
                    pr0, pr1, kc, n, lo = pending.pop(idx)
                    first = idx == 0
                    last = idx == nkc - 1
                    _, vap = kv_aps(kc, hk)
                    # the two row-sum matmuls target different column
                    # groups of the PE array and run concurrently
                    nc.tensor.matmul(ps_s[0:1, lo:], ones_t, pr0,
                                     start=first, stop=last,
                                     skip_group_check=True,
                                     tile_position=(0, 0))
                    nc.tensor.matmul(ps_s[32:33, lo:], ones_t, pr1,
                                     start=first, stop=last,
                                     skip_group_check=True,
                                     tile_position=(0, 32))
                    nc.tensor.matmul(ps_o0[:, lo:], vap, pr0,
                                     start=first, stop=last,
                                     skip_group_check=True)
                    mm_dedup(ps_o1[:, lo:], vap, pr1,
                             start=first, stop=last,
                             skip_group_check=True)
                for h, strip, ps_o in ((h0, 0, ps_o0), (h1, 32, ps_o1)):
                    if phase == "A":
                        # save chunk-A partials; B pass completes them
                        nc.vector.tensor_copy(partA_o[:, h, :], ps_o)
                        nc.vector.tensor_copy(pas(h),
                                              ps_s[strip:strip + 1, :])
                        continue
                    if phase == "F":
                        # single full pass (both chunks were already
                        # gathered when this pair started)
                        sm1 = small.tile([1, RPC], F32, tag="sm1", bufs=2)
                        nc.vector.tensor_copy(sm1, ps_s[strip:strip + 1, :])
                        nc.sync.dma_start(out=sums_dram[h:h + 1, :], in_=sm1)
                        nc.vector.tensor_copy(outu(h), ps_o)
                        continue
                    sm1 = small.tile([1, RPC], F32, tag="sm1", bufs=2)
                    if BLO:
                        nc.vector.tensor_copy(sm1[:, :BLO], pas(h, 0, BLO))
                        nc.vector.tensor_copy(
                            outu_a[:, h, :BLO] if h < 12
                            else outu_b[:, h - 12, :BLO],
                            partA_o[:, h, :BLO])
                    nc.vector.tensor_add(sm1[:, BLO:], pas(h, BLO, RPC),
                                         ps_s[strip:strip + 1, BLO:])
                    nc.sync.dma_start(out=sums_dram[h:h + 1, :], in_=sm1)
                    nc.vector.tensor_add(
                        outu_a[:, h, BLO:] if h < 12
                        else outu_b[:, h - 12, BLO:],
                        partA_o[:, h, BLO:], ps_o[:, BLO:])

            # the first two A pairs interleave with the tail of Q-proj so
            # ScalarE's exp stream starts as soon as chunk A lands
            attn_pair(0, KC_A, "A")
            attn_pair(1, KC_A, "A")
            for oc in range(12, 16):
                qproj(oc)
            unstage(3)
            for hp in range(2, H // 2):
                attn_pair(hp, KC_A, "A")
            for hp in range(H // 2):
                attn_pair(hp, KC_B, "B")
                if hp == 3:
                    normalize_batch(0)
                elif hp == 6:
                    normalize_batch(1)
            normalize_batch(2)

            # ---------------- Phase D: out-projection ----------------
            for oc in range(4):
                wo_all = stream2m.tile([128, H, 512], BF16, tag="s2m")
                nc.sync.dma_start(out=wo_all, in_=wo[oc].rearrange(
                    "p (h m) -> p h m", h=H))
                for qc in range(4):
                    ps_f = psmm.tile([128, 512], F32, tag="mm")
                    for h in range(H):
                        lh = outu_a[:, h, qc * 128:(qc + 1) * 128] if h < 12 \
                            else outu_b[:, h - 12, qc * 128:(qc + 1) * 128]
                        nc.tensor.matmul(
                            ps_f, lh, wo_all[:, h, :],
                            start=(h == 0), stop=(h == H - 1))
                    fin = small.tile([128, 512], F32, tag="fin")
                    nc.vector.tensor_copy(fin, ps_f)
                    nc.sync.dma_start(
                        out=out[qc * 128:(qc + 1) * 128,
                                oc * 512:(oc + 1) * 512],
                        in_=fin)

    nc.compile()
    return nc


def _get_nc(mode: str):
    if mode not in _NC_CACHE:
        _NC_CACHE[mode] = _build(mode)
    return _NC_CACHE[mode]


def _core_rows(mode: str, r: int) -> np.ndarray:
    """Global (within-batch) q-row indices owned by quarter r, ascending.

    causal: 16 interleaved 32-row blocks {4j + r : j} -> exact suffix
    causality, identical shapes on every core.  other modes: contiguous.
    """
    if mode == "causal":
        return np.concatenate([np.arange(32 * (4 * j + r), 32 * (4 * j + r + 1))
                               for j in range(16)])
    return np.arange(r * RPC, (r + 1) * RPC)


def kernel(q, k, v, mask, freqs, W_q, W_k, W_v, W_o):
    q = np.asarray(q, dtype=np.float32)
    k = np.asarray(k, dtype=np.float32)
    v = np.asarray(v, dtype=np.float32)
    mask = np.asarray(mask, dtype=np.float32)
    freqs = np.asarray(freqs, dtype=np.float32)
    W_q = np.asarray(W_q, dtype=np.float32)
    W_k = np.asarray(W_k, dtype=np.float32)
    W_v = np.asarray(W_v, dtype=np.float32)
    W_o = np.asarray(W_o, dtype=np.float32)

    # ---- mask mode detection ----
    nz = mask != 0
    if nz.all():
        mode = "none"
    else:
        tril = np.tril(np.ones((S, S), dtype=bool))
        mode = "causal" if all(np.array_equal(nz[b], tril) for b in range(B)) \
            else "mask"

    # ---- shared host precomputation ----
    c_full = np.cos(freqs)                      # [S, 64]
    s_full = np.sin(freqs)
    sgn = np.tile(np.array([-1.0, 1.0], np.float32), DH // 2)  # [-,+,-,+...]
    cosk_h = np.repeat(c_full, 2, axis=1).T.astype(BF)          # [128, S]
    sink_h = (np.repeat(s_full, 2, axis=1) * sgn).T.astype(BF)

    psw = np.zeros((128, 128), np.float32)
    idx = np.arange(128)
    psw[idx, idx ^ 1] = 1.0
    psw = psw.astype(BF)

    # weight layouts
    # wq[oc, p, i*128+m] = W_q[oc*128+m, i*128+p]
    wq_h = np.ascontiguousarray(
        W_q.reshape(H, 128, NIC, 128).transpose(0, 3, 2, 1)
        .reshape(H, 128, D)).astype(BF)
    # wk[hk, p, i*128+m] = W_k[hk*128+m, i*128+p]
    wk_h = np.ascontiguousarray(
        W_k.reshape(HKV, 128, NIC, 128).transpose(0, 3, 2, 1)
        .reshape(HKV, 128, D)).astype(BF)
    # wv[p, i*512+n] = W_v[n, i*128+p]
    wv_h = np.ascontiguousarray(
        W_v.reshape(DKV, NIC, 128).transpose(2, 1, 0).reshape(128, NIC * DKV)
    ).astype(BF)
    # wo[oc, p, h*512+m] = W_o[oc*512+m, h*128+p]
    wo_h = np.ascontiguousarray(
        W_o.reshape(4, 512, H, 128).transpose(0, 3, 2, 1).reshape(4, 128, -1)
    ).astype(BF)

    # k/v: each core only gets its own 512-token quarter (gathered on device)
    # kt[p, i*512+t] = k[b, tq*512+t, i*128+p] for quarter tq
    kt_b = []   # [B][4] quarters
    vt_b = []
    for b in range(B):
        kt_b.append([np.ascontiguousarray(
            k[b, tq * 512:(tq + 1) * 512].reshape(512, NIC, 128)
            .transpose(2, 1, 0).reshape(128, NIC * 512)).astype(BF)
            for tq in range(4)])
        # vt[j, p, i*128+t] = v[b, tq*512 + j*128+t, i*128+p]
        vt_b.append([np.ascontiguousarray(
            v[b, tq * 512:(tq + 1) * 512].reshape(4, 128, NIC, 128)
            .transpose(0, 3, 2, 1).reshape(4, 128, NIC * 128)).astype(BF)
            for tq in range(4)])

    in_maps = []
    rows_all = []
    for c in range(NCORES):
        b, r = divmod(c, 4)
        rows = _core_rows(mode, r)
        rows_all.append((b, rows))
        # qt[p, i*512+t] = q[b, rows[t], i*128+p]
        qsl = q[b][rows]                       # [512, D]
        qt_h = np.ascontiguousarray(
            qsl.reshape(RPC, NIC, 128).transpose(2, 1, 0).reshape(128, -1)
        ).astype(BF)
        cq = np.repeat(c_full[rows], 2, axis=1).T.astype(BF)      # [128, 512]
        sq = (np.repeat(s_full[rows], 2, axis=1) * sgn).T.astype(BF)
        im = {
            "wq": wq_h, "qt": qt_h, "kt": kt_b[b][r], "vt": vt_b[b][r],
            "wk": wk_h, "wv": wv_h, "wo": wo_h,
            "cosq": cq, "sinq": sq,
            "cosk": np.ascontiguousarray(cosk_h[:, r * 512:(r + 1) * 512]),
            "sink": np.ascontiguousarray(sink_h[:, r * 512:(r + 1) * 512]),
            "pswap": psw,
        }
        if mode == "causal":
            # diagonal 32-col block mask: keep key p of tile kc for local
            # row i of block j=kc  <=>  p <= 32*r + i  (same for all kc)
            pp = np.arange(128)[:, None]
            ii = np.arange(32)[None, :]
            im["mdiag"] = (pp <= 32 * r + ii).astype(BF)
        elif mode == "mask":
            # m01[p, kc*512+m] = (mask[b, rows[m], kc*128+p] != 0)
            msl = nz[b][rows]                  # [512, S] bool
            m01_h = np.ascontiguousarray(
                msl.T.reshape(NKC, 128, RPC).transpose(1, 0, 2)
                .reshape(128, -1)).astype(BF)
            im["m01"] = m01_h
        in_maps.append(im)

    nc = _get_nc(mode)
    kwargs = {}
    if TRACE:
        kwargs["trace"] = True
        if TRACE_CORES:
            kwargs["trace_cores"] = list(TRACE_CORES)
    results = run_bass_kernel_spmd(nc, in_maps, core_ids=list(range(NCORES)),
                                   **kwargs)
    global LAST_RESULTS
    LAST_RESULTS = results

    full = np.empty((B, S, D), np.float32)
    for c in range(NCORES):
        b, rows = rows_all[c]
        full[b, rows] = results.results[c]["out"]
    return full
